# revision 1
# baseline (speedup 1.0000x reference)
"""Trainium2 Bass kernel for the Critic (gnn_message_passing) problem.

Math (per sample b):
  wg   = W_w @ g + W_b                                  [32]
  ul_l = U_w @ x_l + U_b                                [32]  (never materialized)
  score_l = lrelu(a1.wg + a2.ul_l + att_b) = lrelu(x_l . v + c_b)
        where v = U_w^T a2  (128-vec),  c_b = a1.wg + att_b + U_b.a2
  score_g = lrelu((a1+a2).wg + att_b)
  total = score_g + sum_l score_l
  l_part = (U_w @ m_b + U_b * s_b) / total   with m_b = sum_l score_l x_l, s_b = sum_l score_l
  g_part = (score_g / total) * wg
  sa = [relu(g_part); relu(l_part); action]            [128]
  q_h = l3 @ relu(l2 @ relu(l1 @ sa + b1) + b2) + b3   (two heads)

Layout strategy (one NeuronCore handles B_LOC samples, pure data parallel x8):
  - local_states streamed as 128-token x 128-feat tiles (token-partition layout,
    fully contiguous DMA).  t = x.v via DVE tensor_tensor_reduce (c folded in as
    the reduction init); a fraction of tiles computed on GPSIMD to balance.
  - m_b accumulated on PE: lhsT = X_tile (stationary), rhs = [score_lo|score_hi]
    (boundary-masked score columns), accumulating m^T columns per sample in PSUM.
  - s_b via ones-stationary matmuls into a [1, *] PSUM row.
  - Head MLPs run feature-major (transposed activations) on PE.
"""
import os
import sys

sys.path.insert(0, "/opt/trn_rl_repo")

from contextlib import ExitStack

import numpy as np

import concourse.bass as bass
import concourse.tile as tile
from concourse import bacc
from concourse import mybir
from concourse.dve_ops import TENSOR_TENSOR_REDUCE as CUSTOM_TTR

F32 = mybir.dt.float32
AF = mybir.AluOpType

G_DIM, L_DIM, A_DIM, HID = 256, 128, 64, 32
B, L = 4096, 200
NCORES = 8
B_LOC = B // NCORES          # 512 samples per core
PERIOD = 25                  # lcm(200,128)/128 tiles; 16 samples per period
SAMP_PER_PERIOD = 16
GP_CHUNK_SLOTS = {1, 4, 6}   # of every 8 chunks, these run the t-mult on GPSIMD


def _tile_segments(i):
    """Token tile i (128 tokens): samples it touches and the row split."""
    t0 = i * 128
    s0 = t0 // L
    s1 = (t0 + 127) // L
    if s0 == s1:
        return s0, s1, 128
    return s0, s1, L * s1 - t0


def _make_mask_lo(ntile_period=PERIOD):
    m = np.zeros((128, ntile_period), np.float32)
    for j in range(ntile_period):
        _, _, r = _tile_segments(j)
        m[:r, j] = 1.0
    return m


def build_bass(b_loc=B_LOC, block=128, stage="FULL"):
    """Emit the full single-core program. block = samples per PSUM block.
    stage: debug truncation — A, B1a, B1, B, C1, C2, FULL."""
    assert b_loc % SAMP_PER_PERIOD == 0 and block % SAMP_PER_PERIOD == 0
    assert block <= 128 and b_loc % block == 0
    tok = b_loc * L
    ntile = tok // 128
    nchunk = ntile // PERIOD
    tiles_per_block = block * L // 128
    chunks_per_block = tiles_per_block // PERIOD
    assert tiles_per_block % PERIOD == 0

    nc = bacc.Bacc()

    ls = nc.dram_tensor("local_states", [tok, L_DIM], F32, kind="ExternalInput")
    gs = nc.dram_tensor("global_states", [b_loc, G_DIM], F32, kind="ExternalInput")
    ac = nc.dram_tensor("actions", [b_loc, A_DIM], F32, kind="ExternalInput")
    Ww = nc.dram_tensor("W_w", [HID, G_DIM], F32, kind="ExternalInput")
    Wb = nc.dram_tensor("W_b", [HID], F32, kind="ExternalInput")
    Uw = nc.dram_tensor("U_w", [HID, L_DIM], F32, kind="ExternalInput")
    Ub = nc.dram_tensor("U_b", [HID], F32, kind="ExternalInput")
    attw = nc.dram_tensor("att_w", [1, 2 * HID], F32, kind="ExternalInput")
    attb = nc.dram_tensor("att_b", [1], F32, kind="ExternalInput")
    heads = []
    for h, names in enumerate((("l1", "l2", "l3"), ("l4", "l5", "l6"))):
        w1 = nc.dram_tensor(f"{names[0]}_w", [256, 128], F32, kind="ExternalInput")
        b1 = nc.dram_tensor(f"{names[0]}_b", [256], F32, kind="ExternalInput")
        w2 = nc.dram_tensor(f"{names[1]}_w", [256, 256], F32, kind="ExternalInput")
        b2 = nc.dram_tensor(f"{names[1]}_b", [256], F32, kind="ExternalInput")
        w3 = nc.dram_tensor(f"{names[2]}_w", [1, 256], F32, kind="ExternalInput")
        b3 = nc.dram_tensor(f"{names[2]}_b", [1], F32, kind="ExternalInput")
        heads.append((w1, b1, w2, b2, w3, b3))
    mlo = nc.dram_tensor("mask_lo", [128, PERIOD], F32, kind="ExternalInput")
    out_d = nc.dram_tensor("out", [2, b_loc], F32, kind="ExternalOutput")

    nb = b_loc // 128 if b_loc >= 128 else 1   # 128-row groups in b_loc

    with tile.TileContext(nc) as tc, ExitStack() as ctx:
        P = ctx.enter_context(tc.tile_pool(name="persist", bufs=1))
        scratch = ctx.enter_context(tc.tile_pool(name="scratch", bufs=2))
        ctxA = ctx.enter_context(ExitStack())
        ps_t = ctxA.enter_context(tc.tile_pool(name="ps_t", bufs=2, space="PSUM"))

        # ---------------- Phase A: constants & small precompute ----------------
        from concourse.masks import make_identity

        ident = P.tile([128, 128], F32, tag="ident")
        make_identity(nc, ident[:, :])
        zeros128 = P.tile([128, 128], F32, tag="zeros")
        nc.vector.memset(zeros128[:, :], 0.0)
        ones_col = P.tile([128, 1], F32, tag="onesc")
        nc.vector.memset(ones_col[:, :], 1.0)
        ones_row = P.tile([1, 128], F32, tag="onesr")
        nc.vector.memset(ones_row[:, :], 1.0)
        mask_lo = P.tile([128, PERIOD], F32, tag="mlo")
        nc.sync.dma_start(mask_lo[:, :], mlo[:, :])

        def transpose_to_sbuf(dst_ap, src_ap):
            """dst[f, p] = src[p, f] via PE transpose + ACT copy out of PSUM."""
            pp, ff = src_ap.shape
            t_ps = ps_t.tile([128, 128], F32, tag="tps")
            nc.tensor.transpose(t_ps[0:ff, 0:pp], src_ap, ident[0:pp, 0:pp])
            nc.scalar.copy(dst_ap, t_ps[0:ff, 0:pp])

        # small weights
        Ww_sb = P.tile([HID, G_DIM], F32, tag="Ww")
        nc.sync.dma_start(Ww_sb[:, :], Ww[:, :])
        Wb_sb = P.tile([HID, 1], F32, tag="Wb")
        nc.sync.dma_start(Wb_sb[:, :], Wb[:][:, None])
        Uw_sb = P.tile([HID, L_DIM], F32, tag="Uw")
        nc.sync.dma_start(Uw_sb[:, :], Uw[:, :])
        Ub_col = P.tile([HID, 1], F32, tag="Ubc")
        nc.sync.dma_start(Ub_col[:, :], Ub[:][:, None])
        Ub_row = P.tile([1, HID], F32, tag="Ubr")
        nc.sync.dma_start(Ub_row[:, :], Ub[:][None, :])
        a1_sb = P.tile([HID, 1], F32, tag="a1")
        nc.sync.dma_start(a1_sb[:, :], attw[0, 0:HID][:, None])
        a2_sb = P.tile([HID, 1], F32, tag="a2")
        nc.sync.dma_start(a2_sb[:, :], attw[0, HID:2 * HID][:, None])
        attb_sb = P.tile([1, 1], F32, tag="attb")
        nc.sync.dma_start(attb_sb[:, :], attb[:][None, :])

        WwT = []  # W_w^T in [128, HID] chunks over G_DIM
        for g in range(G_DIM // 128):
            w = P.tile([128, HID], F32, tag=f"WwT{g}")
            transpose_to_sbuf(w[:, :], Ww_sb[:, g * 128:(g + 1) * 128])
            WwT.append(w)
        UwT = P.tile([L_DIM, HID], F32, tag="UwT")
        transpose_to_sbuf(UwT[:, :], Uw_sb[:, :])

        # gT: global_states^T  [G_DIM partition-chunks][128, b_loc]
        gT = []
        for g in range(G_DIM // 128):
            t = P.tile([128, b_loc], F32, tag=f"gT{g}")
            gT.append(t)
        for bb in range(nb):
            bs = min(128, b_loc)
            g_nat = scratch.tile([128, G_DIM], F32, tag="gnat")
            nc.sync.dma_start(g_nat[0:bs, :], gs[bb * 128:bb * 128 + bs, :])
            for g in range(G_DIM // 128):
                transpose_to_sbuf(gT[g][:, bb * 128:bb * 128 + bs],
                                  g_nat[0:bs, g * 128:(g + 1) * 128])

        # wg^T [HID, b_loc]
        wgT_ps = ps_t.tile([HID, b_loc], F32, tag="tps")
        for g in range(G_DIM // 128):
            nc.tensor.matmul(out=wgT_ps[:, :], lhsT=WwT[g][:, :], rhs=gT[g][:, :],
                             start=(g == 0), stop=(g == G_DIM // 128 - 1))
        wgT = P.tile([HID, b_loc], F32, tag="wgT")
        nc.scalar.activation(wgT[:, :], wgT_ps[:, :],
                             mybir.ActivationFunctionType.Identity, bias=Wb_sb[:, :])

        # v_row [1, 128] = a2^T U_w ;  v_rep [128, 128] = ones ⊗ v_row
        v_ps = ps_t.tile([1, L_DIM], F32, tag="tps")
        nc.tensor.matmul(out=v_ps[:, :], lhsT=a2_sb[:, :], rhs=Uw_sb[:, :])
        v_row = P.tile([1, L_DIM], F32, tag="vrow")
        nc.scalar.copy(v_row[:, :], v_ps[:, :])
        vrep_ps = ps_t.tile([128, 128], F32, tag="tps")
        nc.tensor.matmul(out=vrep_ps[:, :], lhsT=ones_row[:, :], rhs=v_row[:, :])
        v_rep = P.tile([128, 128], F32, tag="vrep")
        nc.scalar.copy(v_rep[:, :], vrep_ps[:, :])

        # c_row [1, b_loc] = a1.wg + att_b + U_b.a2 ; sg_raw = lrelu((a1+a2).wg + att_b)
        uba2_ps = ps_t.tile([1, 1], F32, tag="tps")
        nc.tensor.matmul(out=uba2_ps[:, :], lhsT=Ub_col[:, :], rhs=a2_sb[:, :])
        cconst = P.tile([1, 1], F32, tag="cconst")
        nc.vector.tensor_tensor(out=cconst[:, :], in0=uba2_ps[:, :], in1=attb_sb[:, :],
                                op=AF.add)
        c_ps = ps_t.tile([1, b_loc], F32, tag="tps")
        nc.tensor.matmul(out=c_ps[:, :], lhsT=a1_sb[:, :], rhs=wgT[:, :])
        c_row = P.tile([1, b_loc], F32, tag="crow")
        nc.scalar.activation(c_row[:, :], c_ps[:, :],
                             mybir.ActivationFunctionType.Identity, bias=cconst[:, :])

        a12 = P.tile([HID, 1], F32, tag="a12")
        nc.vector.tensor_tensor(out=a12[:, :], in0=a1_sb[:, :], in1=a2_sb[:, :],
                                op=AF.add)
        sg_ps = ps_t.tile([1, b_loc], F32, tag="tps")
        nc.tensor.matmul(out=sg_ps[:, :], lhsT=a12[:, :], rhs=wgT[:, :])
        sg_lin = P.tile([1, b_loc], F32, tag="sg_lin")
        nc.scalar.activation(sg_lin[:, :], sg_ps[:, :],
                             mybir.ActivationFunctionType.Identity, bias=attb_sb[:, :])
        sg_raw = P.tile([1, b_loc], F32, tag="sg_raw")
        nc.vector.scalar_tensor_tensor(out=sg_raw[:, :], in0=sg_lin[:, :], scalar=0.01,
                                       in1=sg_lin[:, :], op0=AF.mult, op1=AF.max)

        # c_rep [128, b_loc] then c_sel [128, ntile]
        crep_ps = ps_t.tile([128, b_loc], F32, tag="tps")
        nc.tensor.matmul(out=crep_ps[:, :], lhsT=ones_row[:, :], rhs=c_row[:, :])
        c_rep = P.tile([128, b_loc], F32, tag="crep")
        nc.scalar.copy(c_rep[:, :], crep_ps[:, :])
        ngrp = b_loc // SAMP_PER_PERIOD  # periods in b_loc
        c_sel = P.tile([128, ntile], F32, tag="csel")
        cdiff = scratch.tile([128, ngrp], F32, tag="cdiff")
        for j in range(PERIOD):
            s0, s1, r = _tile_segments(j)
            c_lo = c_rep[:, s0:b_loc:SAMP_PER_PERIOD]
            if s0 == s1:
                nc.vector.tensor_copy(c_sel[:, j:ntile:PERIOD], c_lo)
            else:
                # rows < r take c[s0], rows >= r take c[s1]:
                #   c_sel = (c_lo - c_hi) * mask_lo[:, j] + c_hi
                c_hi = c_rep[:, s1:b_loc:SAMP_PER_PERIOD]
                nc.vector.tensor_tensor(out=cdiff[:, :], in0=c_lo, in1=c_hi,
                                        op=AF.subtract)
                nc.vector.scalar_tensor_tensor(
                    out=c_sel[:, j:ntile:PERIOD], in0=cdiff[:, :],
                    scalar=mask_lo[:, j:j + 1], in1=c_hi,
                    op0=AF.mult, op1=AF.add)

        c128 = P.tile([128, ntile], F32, tag="c128")
        nc.vector.tensor_scalar_mul(c128[:, :], c_sel[:, :], 1.0 / 128.0)

        # actions^T into sa^T[64:128]
        saT = P.tile([128, b_loc], F32, tag="saT")
        for bb in range(nb):
            bs = min(128, b_loc)
            a_nat = scratch.tile([128, A_DIM], F32, tag="anat")
            nc.sync.dma_start(a_nat[0:bs, :], ac[bb * 128:bb * 128 + bs, :])
            transpose_to_sbuf(saT[2 * HID:2 * HID + A_DIM, bb * 128:bb * 128 + bs],
                              a_nat[0:bs, :])

        # MLP head weights, transposed
        head_sb = []
        for (w1, b1, w2, b2, w3, b3) in heads:
            w1_nat = scratch.tile([128, 128], F32, tag="w1nat")
            w1T = P.tile([128, 256], F32, tag=f"w1T{len(head_sb)}")
            for rh in range(2):
                nc.sync.dma_start(w1_nat[:, :], w1[rh * 128:(rh + 1) * 128, :])
                transpose_to_sbuf(w1T[:, rh * 128:(rh + 1) * 128], w1_nat[:, :])
            w2T = [P.tile([128, 256], F32, tag=f"w2T{len(head_sb)}_{kh}",
                          name=f"w2T{len(head_sb)}_{kh}")
                   for kh in range(2)]
            for rh in range(2):
                for kh in range(2):
                    w2_nat = scratch.tile([128, 128], F32, tag="w2nat")
                    nc.sync.dma_start(
                        w2_nat[:, :],
                        w2[rh * 128:(rh + 1) * 128, kh * 128:(kh + 1) * 128])
                    transpose_to_sbuf(w2T[kh][:, rh * 128:(rh + 1) * 128],
                                      w2_nat[:, :])
            w3T = P.tile([128, 2], F32, tag=f"w3T{len(head_sb)}")
            for kh in range(2):
                nc.sync.dma_start(w3T[:, kh:kh + 1],
                                  w3[0, kh * 128:(kh + 1) * 128][:, None])
            b1c = P.tile([128, 2], F32, tag=f"b1c{len(head_sb)}")
            b2c = P.tile([128, 2], F32, tag=f"b2c{len(head_sb)}")
            for rh in range(2):
                nc.sync.dma_start(b1c[:, rh:rh + 1],
                                  b1[rh * 128:(rh + 1) * 128][:, None])
                nc.sync.dma_start(b2c[:, rh:rh + 1],
                                  b2[rh * 128:(rh + 1) * 128][:, None])
            b3c = P.tile([1, 1], F32, tag=f"b3c{len(head_sb)}")
            nc.sync.dma_start(b3c[:, :], b3[:][None, :])
            head_sb.append((w1T, w2T, w3T, b1c, b2c, b3c))

        ctxA.close()

        # ---------------- Phase B: main token stream ----------------
        ctxB = ctx.enter_context(ExitStack())
        xpool = ctx.enter_context(tc.tile_pool(name="xchunk", bufs=3))
        tpool = ctx.enter_context(tc.tile_pool(name="tbuf", bufs=2))
        s2pool = ctx.enter_context(tc.tile_pool(name="score2", bufs=2))
        jpool = ctx.enter_context(tc.tile_pool(name="junk", bufs=2))
        ppool = ctx.enter_context(tc.tile_pool(name="prod", bufs=2))
        ps_m = ctxB.enter_context(tc.tile_pool(name="ps_m", bufs=2, space="PSUM"))
        ps_s = ctxB.enter_context(tc.tile_pool(name="ps_s", bufs=2, space="PSUM"))

        mT = P.tile([L_DIM, b_loc], F32, tag="mT")
        s_row = P.tile([1, b_loc], F32, tag="srow")

        ls_flat = ls[:, :]
        m_ps = None
        s_ps = None
        for ch in range(nchunk):
            x_ch = xpool.tile([128, PERIOD * 128], F32, tag="xch")
            src = ls_flat[ch * PERIOD * 128:(ch + 1) * PERIOD * 128, :]
            nc.sync.dma_start(
                x_ch[:, :].rearrange("p (j d) -> p j d", d=L_DIM),
                src.rearrange("(j p) d -> p j d", p=128))

            blk = (ch * PERIOD) // tiles_per_block
            if stage == 'A':
                continue
            if ch % chunks_per_block == 0 and stage not in ('B1a', 'B1'):
                m_ps = ps_m.tile([L_DIM, block + 1], F32, tag="mps")
                s_ps = ps_s.tile([1, block + 1], F32, tag="sps")
                nc.tensor.matmul(out=m_ps[:, :], lhsT=zeros128[:, 0:L_DIM],
                                 rhs=x_ch[:, 0:block + 1], start=True, stop=False,
                                 skip_group_check=True)
                nc.tensor.matmul(out=s_ps[:, :], lhsT=zeros128[:, 0:1],
                                 rhs=x_ch[:, 0:block + 1], start=True, stop=False,
                                 skip_group_check=True)

            # t-pass: DVE chunks use fused tensor_tensor_reduce (c as init);
            # GPSIMD chunks do one big elementwise mult, ACT reduces per tile
            # with bias=c/128 folded into the accumulated sum.
            t_buf = tpool.tile([128, PERIOD], F32, tag="tb")
            use_gp = (ch % 8) in GP_CHUNK_SLOTS
            junk = jpool.tile([128, 128], F32, tag="jk")
            if use_gp:
                prod = ppool.tile([128, PERIOD * 128], F32, tag="pr")
                nc.gpsimd.tensor_tensor(
                    out=prod[:, :].rearrange("p (j d) -> p j d", d=128),
                    in0=x_ch[:, :].rearrange("p (j d) -> p j d", d=128),
                    in1=v_rep[:, None, :].broadcast_to((128, PERIOD, 128)),
                    op=AF.mult)
                for j in range(PERIOD):
                    i = ch * PERIOD + j
                    nc.scalar.activation(
                        junk[:, :], prod[:, j * 128:(j + 1) * 128],
                        mybir.ActivationFunctionType.Identity,
                        bias=c128[:, i:i + 1], accum_out=t_buf[:, j:j + 1])
            else:
                for j in range(PERIOD):
                    i = ch * PERIOD + j
                    nc.vector._custom_dve(
                        CUSTOM_TTR, out=junk[:, :],
                        in0=x_ch[:, j * 128:(j + 1) * 128], in1=v_rep[:, :],
                        s0=c_sel[:, i:i + 1], s1=1.0,
                        accum_out=t_buf[:, j:j + 1])

            # score + masked lo/hi columns
            if stage == 'B1a':
                continue
            score2 = s2pool.tile([128, 2 * PERIOD], F32, tag="s2")
            sc2 = score2[:, :].rearrange("p (j two) -> p j two", two=2)
            score = tpool.tile([128, PERIOD], F32, tag="sc")
            nc.vector.scalar_tensor_tensor(
                out=score[:, :], in0=t_buf[:, :], scalar=0.01, in1=t_buf[:, :],
                op0=AF.mult, op1=AF.max)
            nc.vector.tensor_tensor(out=sc2[:, :, 0], in0=score[:, :],
                                    in1=mask_lo[:, :], op=AF.mult)
            nc.vector.tensor_tensor(out=sc2[:, :, 1], in0=score[:, :],
                                    in1=sc2[:, :, 0], op=AF.subtract)

            # m accumulation (X stationary), then s burst (ones stationary)
            if stage == 'B1':
                continue
            last_in_block = (ch % chunks_per_block) == chunks_per_block - 1
            for j in range(PERIOD):
                i = ch * PERIOD + j
                col = (i * 128) // L - blk * block
                stop = last_in_block and j == PERIOD - 1
                nc.tensor.matmul(out=m_ps[:, col:col + 2],
                                 lhsT=x_ch[:, j * 128:(j + 1) * 128],
                                 rhs=score2[:, 2 * j:2 * j + 2],
                                 start=False, stop=stop, skip_group_check=True)
            for j in range(PERIOD):
                i = ch * PERIOD + j
                col = (i * 128) // L - blk * block
                stop = last_in_block and j == PERIOD - 1
                nc.tensor.matmul(out=s_ps[:, col:col + 2], lhsT=ones_col[:, :],
                                 rhs=score2[:, 2 * j:2 * j + 2],
                                 start=False, stop=stop, skip_group_check=True)

            if last_in_block:
                nc.scalar.copy(mT[:, blk * block:(blk + 1) * block],
                               m_ps[:, 0:block])
                nc.scalar.copy(s_row[:, blk * block:(blk + 1) * block],
                               s_ps[:, 0:block])

        ctxB.close()

        # ---------------- Phase C: combine + heads ----------------
        do_c = stage not in ('A', 'B1a', 'B1', 'B')
        ps_c = (ctx.enter_context(tc.tile_pool(name="ps_c", bufs=4, space="PSUM"))
                if do_c else None)
        if do_c:
            _phase_c(nc, tc, ctx, stage, b_loc, P, scratch, ps_c, sg_raw, s_row,
                     ones_row, UwT, mT, Ub_row, wgT, saT, head_sb, out_d)

    nc.compile()
    return nc


def _phase_c(nc, tc, ctx, stage, b_loc, P, scratch, ps_c, sg_raw, s_row,
             ones_row, UwT, mT, Ub_row, wgT, saT, head_sb, out_d):
    if True:

        total = P.tile([1, b_loc], F32, tag="total")
        nc.vector.tensor_tensor(out=total[:, :], in0=sg_raw[:, :], in1=s_row[:, :],
                                op=AF.add)
        recip = P.tile([1, b_loc], F32, tag="recip")
        nc.vector.reciprocal(recip[:, :], total[:, :])
        gn_row = P.tile([1, b_loc], F32, tag="gn")
        nc.vector.tensor_tensor(out=gn_row[:, :], in0=sg_raw[:, :], in1=recip[:, :],
                                op=AF.mult)
        if stage == 'C1':
            nc.sync.dma_start(out_d[0:1, :], gn_row[:, :])
            return

        r32_ps = ps_c.tile([HID, b_loc], F32, tag="cps")
        nc.tensor.matmul(out=r32_ps[:, :], lhsT=ones_row[0:1, 0:HID], rhs=recip[:, :])
        r32 = P.tile([HID, b_loc], F32, tag="r32")
        nc.scalar.copy(r32[:, :], r32_ps[:, :])
        g32_ps = ps_c.tile([HID, b_loc], F32, tag="cps")
        nc.tensor.matmul(out=g32_ps[:, :], lhsT=ones_row[0:1, 0:HID], rhs=gn_row[:, :])
        g32 = P.tile([HID, b_loc], F32, tag="g32")
        nc.scalar.copy(g32[:, :], g32_ps[:, :])

        lT_ps = ps_c.tile([HID, b_loc], F32, tag="cps")
        nc.tensor.matmul(out=lT_ps[:, :], lhsT=UwT[:, :], rhs=mT[:, :],
                         start=True, stop=False)
        nc.tensor.matmul(out=lT_ps[:, :], lhsT=Ub_row[:, :], rhs=s_row[:, :],
                         start=False, stop=True)

        lnorm = P.tile([HID, b_loc], F32, tag="lnorm")
        nc.vector.tensor_tensor(out=lnorm[:, :], in0=lT_ps[:, :], in1=r32[:, :],
                                op=AF.mult)
        gpart = P.tile([HID, b_loc], F32, tag="gpart")
        nc.vector.tensor_tensor(out=gpart[:, :], in0=wgT[:, :], in1=g32[:, :],
                                op=AF.mult)
        nc.scalar.activation(saT[0:HID, :], gpart[:, :],
                             mybir.ActivationFunctionType.Relu)
        nc.scalar.activation(saT[HID:2 * HID, :], lnorm[:, :],
                             mybir.ActivationFunctionType.Relu)
        if stage == 'C2':
            nc.sync.dma_start(out_d[0:1, 0:HID], lnorm[0:1, 0:HID])
            return

        for h, (w1T, w2T, w3T, b1c, b2c, b3c) in enumerate(head_sb):
            h1 = []
            for rh in range(2):
                h_ps = ps_c.tile([128, b_loc], F32, tag="cps")
                nc.tensor.matmul(out=h_ps[:, :], lhsT=w1T[:, rh * 128:(rh + 1) * 128],
                                 rhs=saT[:, :])
                h_sb = scratch.tile([128, b_loc], F32, tag="h1sb")
                nc.scalar.activation(h_sb[:, :], h_ps[:, :],
                                     mybir.ActivationFunctionType.Relu,
                                     bias=b1c[:, rh:rh + 1])
                h1.append(h_sb)
            h2 = []
            for rh in range(2):
                h_ps = ps_c.tile([128, b_loc], F32, tag="cps")
                for kh in range(2):
                    nc.tensor.matmul(out=h_ps[:, :],
                                     lhsT=w2T[kh][:, rh * 128:(rh + 1) * 128],
                                     rhs=h1[kh][:, :],
                                     start=(kh == 0), stop=(kh == 1))
                h_sb = scratch.tile([128, b_loc], F32, tag="h2sb")
                nc.scalar.activation(h_sb[:, :], h_ps[:, :],
                                     mybir.ActivationFunctionType.Relu,
                                     bias=b2c[:, rh:rh + 1])
                h2.append(h_sb)
            q_ps = ps_c.tile([1, b_loc], F32, tag="cps")
            for kh in range(2):
                nc.tensor.matmul(out=q_ps[:, :], lhsT=w3T[:, kh:kh + 1],
                                 rhs=h2[kh][:, :], start=(kh == 0), stop=(kh == 1))
            q_row = scratch.tile([1, b_loc], F32, tag="qrow")
            nc.scalar.activation(q_row[:, :], q_ps[:, :],
                                 mybir.ActivationFunctionType.Identity,
                                 bias=b3c[:, :])
            nc.sync.dma_start(out_d[h:h + 1, :], q_row[:, :])


def _shard_inputs(inputs, b_loc=B_LOC):
    """Full inputs -> list of per-core in_maps."""
    mask = _make_mask_lo()
    maps = []
    for c in range(NCORES):
        sl = slice(c * b_loc, (c + 1) * b_loc)
        m = {
            "local_states": np.ascontiguousarray(
                inputs["local_states"][sl].reshape(b_loc * L, L_DIM)),
            "global_states": np.ascontiguousarray(inputs["global_states"][sl]),
            "actions": np.ascontiguousarray(inputs["actions"][sl]),
            "mask_lo": mask,
        }
        for k in ("W_w", "W_b", "U_w", "U_b", "att_b",
                  "l1_w", "l1_b", "l2_w", "l2_b", "l3_w", "l3_b",
                  "l4_w", "l4_b", "l5_w", "l5_b", "l6_w", "l6_b"):
            m[k] = np.ascontiguousarray(np.asarray(inputs[k], np.float32))
        m["att_w"] = np.ascontiguousarray(
            np.asarray(inputs["att_w"], np.float32).reshape(1, 2 * HID))
        maps.append(m)
    return maps


_CACHE = {}


def kernel(**inputs) -> np.ndarray:
    from concourse.bass_utils import run_bass_kernel_spmd

    inputs = {k: np.asarray(v, np.float32) for k, v in inputs.items()}
    if "nc" not in _CACHE:
        _CACHE["nc"] = build_bass()
    nc = _CACHE["nc"]
    maps = _shard_inputs(inputs)
    res = run_bass_kernel_spmd(nc, maps, list(range(NCORES)))
    outs = [res.results[c]["out"] for c in range(NCORES)]  # each [2, B_LOC]
    q = np.concatenate(outs, axis=1)  # [2, B]
    return q.reshape(2, B, 1).astype(np.float32)



# revision 6
# speedup vs baseline: 1.3224x; 1.3224x over previous
"""Trainium2 Bass kernel for the Critic (gnn_message_passing) problem.

Math (per sample b):
  wg   = W_w @ g + W_b                                  [32]
  score_l = lrelu(x_l . v + c_b),  v = U_w^T a2,  c_b = a1.wg + att_b + U_b.a2
  score_g = lrelu((a1+a2).wg + att_b)
  total = score_g + sum_l score_l
  l_part = (U_w @ m_b + U_b * s_b) / total   with m_b = sum_l score_l x_l, s_b = sum_l score_l
  g_part = (score_g / total) * wg
  sa = [relu(g_part); relu(l_part); action]            [128]
  q_h = l3 @ relu(l2 @ relu(l1 @ sa + b1) + b2) + b3   (two heads)

Layout strategy (one NeuronCore handles B_LOC=512 samples, pure data parallel x8):
  Tokens are laid out BLOCKED: within a 16-sample chunk (3200 tokens), SBUF
  partition p holds tokens [25p, 25p+25).  Since 200 = 8*25, partition p
  always belongs to sample p//8 - no sample straddles a partition, so the
  attention bias c is a per-partition scalar and no boundary masking exists.
  DMA moves 12.8KB contiguous per partition per chunk.

  Per chunk (25 column-tiles of [128, 128]):
   - mult x*v: one big GPSIMD op (tiles 0:G) + one big DVE op (tiles G:25)
   - reduce to t[128,25]: one segmented DVE tensor_reduce (tiles 0:R) +
     per-tile ACT accum reduces (tiles R:25)
   - score = Lrelu(t + c) in one ACT op (c per-partition bias)
   - S_all[p,t',g] = score[p,t'] * [p//8==g]: one DVE mult with the
     block mask; lhsT of the m-matmul.
   - m^T[16,129] accumulated in PSUM over 25 matmuls: lhsT=S_all[:,t',:]
     (16 cols, cheap LDWEIGHTS), rhs=[x_tile | ones] (s_b rides along as
     column 128).
  At the end m (sample-major) is PE-transposed to feature-major mT for the
  combine; heads run feature-major on PE as in the reference mapping.
"""
import os
import sys

sys.path.insert(0, "/opt/trn_rl_repo")

from contextlib import ExitStack

import numpy as np

import concourse.bass as bass
import concourse.tile as tile
from concourse import bacc
from concourse import mybir

F32 = mybir.dt.float32
AF = mybir.AluOpType
ACTF = mybir.ActivationFunctionType

G_DIM, L_DIM, A_DIM, HID = 256, 128, 64, 32
B, L = 4096, 200
NCORES = 8
B_LOC = B // NCORES          # 512 samples per core
PERIOD = 25                  # column-tiles per chunk; 16 samples per chunk
SAMP = 16                    # samples per chunk
XW = 132                     # x_ch row stride in elems (129 used, pad 3)
G_TILES = 16                 # tiles whose x*v mult runs on GPSIMD
R_TILES = 19                 # tiles reduced by the segmented DVE reduce


def _make_gb():
    gb = np.zeros((128, SAMP), np.float32)
    for p in range(128):
        gb[p, p // 8] = 1.0
    return gb


def _make_perm8():
    # perm[q, 128k+p] = 1 iff q == 16k + p//8; used as lhsT so that
    # (perm_k)^T @ c_colblk gives c_all[p] = c[16k + p//8]
    m = np.zeros((128, 8 * 128), np.float32)
    for k in range(8):
        for p in range(128):
            m[16 * k + p // 8, 128 * k + p] = 1.0
    return m


def build_bass(b_loc=B_LOC):
    tok = b_loc * L
    nchunk = tok // (PERIOD * 128)     # 32
    nblk = nchunk // 8                 # 4 sample-blocks of 128

    nc = bacc.Bacc()

    ls = nc.dram_tensor("local_states", [tok, L_DIM], F32, kind="ExternalInput")
    gs = nc.dram_tensor("global_states", [b_loc, G_DIM], F32, kind="ExternalInput")
    ac = nc.dram_tensor("actions", [b_loc, A_DIM], F32, kind="ExternalInput")
    Ww = nc.dram_tensor("W_w", [HID, G_DIM], F32, kind="ExternalInput")
    Wb = nc.dram_tensor("W_b", [HID], F32, kind="ExternalInput")
    Uw = nc.dram_tensor("U_w", [HID, L_DIM], F32, kind="ExternalInput")
    Ub = nc.dram_tensor("U_b", [HID], F32, kind="ExternalInput")
    attw = nc.dram_tensor("att_w", [1, 2 * HID], F32, kind="ExternalInput")
    attb = nc.dram_tensor("att_b", [1], F32, kind="ExternalInput")
    heads = []
    for h, names in enumerate((("l1", "l2", "l3"), ("l4", "l5", "l6"))):
        w1 = nc.dram_tensor(f"{names[0]}_w", [256, 128], F32, kind="ExternalInput")
        b1 = nc.dram_tensor(f"{names[0]}_b", [256], F32, kind="ExternalInput")
        w2 = nc.dram_tensor(f"{names[1]}_w", [256, 256], F32, kind="ExternalInput")
        b2 = nc.dram_tensor(f"{names[1]}_b", [256], F32, kind="ExternalInput")
        w3 = nc.dram_tensor(f"{names[2]}_w", [1, 256], F32, kind="ExternalInput")
        b3 = nc.dram_tensor(f"{names[2]}_b", [1], F32, kind="ExternalInput")
        heads.append((w1, b1, w2, b2, w3, b3))
    gb_d = nc.dram_tensor("gb", [128, SAMP], F32, kind="ExternalInput")
    perm_d = nc.dram_tensor("perm8", [128, 8 * 128], F32, kind="ExternalInput")
    out_d = nc.dram_tensor("out", [2, b_loc], F32, kind="ExternalOutput")

    nb = b_loc // 128

    with tile.TileContext(nc) as tc, ExitStack() as ctx:
        P = ctx.enter_context(tc.tile_pool(name="persist", bufs=1))
        scratch = ctx.enter_context(tc.tile_pool(name="scratch", bufs=2))
        ctxA = ctx.enter_context(ExitStack())
        ps_t = ctxA.enter_context(tc.tile_pool(name="ps_t", bufs=2, space="PSUM"))

        # ---------------- Phase A: constants & small precompute ----------------
        from concourse.masks import make_identity

        ident = P.tile([128, 128], F32, tag="ident")
        make_identity(nc, ident[:, :])
        ones_row = P.tile([1, 128], F32, tag="onesr")
        nc.vector.memset(ones_row[:, :], 1.0)
        gb_sb = P.tile([128, SAMP], F32, tag="gb")
        nc.sync.dma_start(gb_sb[:, :], gb_d[:, :])
        perm_sb = P.tile([128, 8 * 128], F32, tag="perm")
        nc.sync.dma_start(perm_sb[:, :], perm_d[:, :])

        def transpose_to_sbuf(dst_ap, src_ap):
            """dst[f, p] = src[p, f] via PE transpose + ACT copy out of PSUM."""
            pp, ff = src_ap.shape
            t_ps = ps_t.tile([128, 128], F32, tag="tps")
            nc.tensor.transpose(t_ps[0:ff, 0:pp], src_ap, ident[0:pp, 0:pp])
            nc.scalar.copy(dst_ap, t_ps[0:ff, 0:pp])

        # small weights
        Ww_sb = P.tile([HID, G_DIM], F32, tag="Ww")
        nc.sync.dma_start(Ww_sb[:, :], Ww[:, :])
        Wb_sb = P.tile([HID, 1], F32, tag="Wb")
        nc.sync.dma_start(Wb_sb[:, :], Wb[:][:, None])
        Uw_sb = P.tile([HID, L_DIM], F32, tag="Uw")
        nc.sync.dma_start(Uw_sb[:, :], Uw[:, :])
        Ub_col = P.tile([HID, 1], F32, tag="Ubc")
        nc.sync.dma_start(Ub_col[:, :], Ub[:][:, None])
        Ub_row = P.tile([1, HID], F32, tag="Ubr")
        nc.sync.dma_start(Ub_row[:, :], Ub[:][None, :])
        a1_sb = P.tile([HID, 1], F32, tag="a1")
        nc.sync.dma_start(a1_sb[:, :], attw[0, 0:HID][:, None])
        a2_sb = P.tile([HID, 1], F32, tag="a2")
        nc.sync.dma_start(a2_sb[:, :], attw[0, HID:2 * HID][:, None])
        attb_sb = P.tile([1, 1], F32, tag="attb")
        nc.sync.dma_start(attb_sb[:, :], attb[:][None, :])

        WwT = []  # W_w^T in [128, HID] chunks over G_DIM
        for g in range(G_DIM // 128):
            w = P.tile([128, HID], F32, tag=f"WwT{g}")
            transpose_to_sbuf(w[:, :], Ww_sb[:, g * 128:(g + 1) * 128])
            WwT.append(w)
        UwT = P.tile([L_DIM, HID], F32, tag="UwT")
        transpose_to_sbuf(UwT[:, :], Uw_sb[:, :])

        # gT: global_states^T  [G_DIM partition-chunks][128, b_loc]
        gT = []
        for g in range(G_DIM // 128):
            t = P.tile([128, b_loc], F32, tag=f"gT{g}")
            gT.append(t)
        for bb in range(nb):
            g_nat = scratch.tile([128, G_DIM], F32, tag="gnat")
            nc.sync.dma_start(g_nat[:, :], gs[bb * 128:(bb + 1) * 128, :])
            for g in range(G_DIM // 128):
                transpose_to_sbuf(gT[g][:, bb * 128:(bb + 1) * 128],
                                  g_nat[:, g * 128:(g + 1) * 128])

        # wg^T [HID, b_loc]
        wgT_ps = ps_t.tile([HID, b_loc], F32, tag="wps")
        for g in range(G_DIM // 128):
            nc.tensor.matmul(out=wgT_ps[:, :], lhsT=WwT[g][:, :], rhs=gT[g][:, :],
                             start=(g == 0), stop=(g == G_DIM // 128 - 1))
        wgT = P.tile([HID, b_loc], F32, tag="wgT")
        nc.scalar.activation(wgT[:, :], wgT_ps[:, :], ACTF.Identity, bias=Wb_sb[:, :])

        # v_row [1, 128] = a2^T U_w ;  v_rep [128, 128] = ones (x) v_row
        v_ps = ps_t.tile([1, L_DIM], F32, tag="tps")
        nc.tensor.matmul(out=v_ps[:, :], lhsT=a2_sb[:, :], rhs=Uw_sb[:, :])
        v_row = P.tile([1, L_DIM], F32, tag="vrow")
        nc.scalar.copy(v_row[:, :], v_ps[:, :])
        vrep_ps = ps_t.tile([128, 128], F32, tag="tps")
        nc.tensor.matmul(out=vrep_ps[:, :], lhsT=ones_row[:, :], rhs=v_row[:, :])
        v_rep = P.tile([128, 128], F32, tag="vrep")
        nc.scalar.copy(v_rep[:, :], vrep_ps[:, :])

        # cconst = Ub.a2 + att_b, replicated to [128, 1]
        uba2_ps = ps_t.tile([1, 1], F32, tag="tps")
        nc.tensor.matmul(out=uba2_ps[:, :], lhsT=Ub_col[:, :], rhs=a2_sb[:, :])
        cconst = P.tile([1, 1], F32, tag="cconst")
        nc.vector.tensor_tensor(out=cconst[:, :], in0=uba2_ps[:, :], in1=attb_sb[:, :],
                                op=AF.add)
        ccol_ps = ps_t.tile([128, 1], F32, tag="tps")
        nc.tensor.matmul(out=ccol_ps[:, :], lhsT=ones_row[:, :], rhs=cconst[:, :])
        cconst_col = P.tile([128, 1], F32, tag="cconstc")
        nc.scalar.copy(cconst_col[:, :], ccol_ps[:, :])

        # c_colblk [128, nblk]: c_lin of sample 128b+q at (q, b) (no const yet)
        c_colblk = P.tile([128, nblk], F32, tag="ccb")
        for b in range(nblk):
            cb_ps = ps_t.tile([128, 1], F32, tag="tps")
            nc.tensor.matmul(out=cb_ps[:, :], lhsT=wgT[:, b * 128:(b + 1) * 128],
                             rhs=a1_sb[:, :])
            nc.scalar.copy(c_colblk[:, b:b + 1], cb_ps[:, :])

        # c_all [128, nchunk]: c (incl const) for partition p in chunk ch
        call_ps = ps_t.tile([128, nchunk], F32, tag="cps2")
        for k in range(8):
            nc.tensor.matmul(out=call_ps[:, k:nchunk:8],
                             lhsT=perm_sb[:, k * 128:(k + 1) * 128],
                             rhs=c_colblk[:, :], start=True, stop=True,
                             skip_group_check=True)
        c_all = P.tile([128, nchunk], F32, tag="call")
        nc.scalar.activation(c_all[:, :], call_ps[:, :], ACTF.Identity,
                             bias=cconst_col[:, :])

        # sg_raw [1, b_loc] = lrelu((a1+a2).wg + att_b)
        a12 = P.tile([HID, 1], F32, tag="a12")
        nc.vector.tensor_tensor(out=a12[:, :], in0=a1_sb[:, :], in1=a2_sb[:, :],
                                op=AF.add)
        sg_ps = ps_t.tile([1, b_loc], F32, tag="tps")
        nc.tensor.matmul(out=sg_ps[:, :], lhsT=a12[:, :], rhs=wgT[:, :])
        sg_raw = P.tile([1, b_loc], F32, tag="sg_raw")
        nc.scalar.activation(sg_raw[:, :], sg_ps[:, :], ACTF.Lrelu,
                             bias=attb_sb[:, :], alpha=0.01)

        # actions^T into sa^T[64:128]
        saT = P.tile([128, b_loc], F32, tag="saT")
        for bb in range(nb):
            a_nat = scratch.tile([128, A_DIM], F32, tag="anat")
            nc.sync.dma_start(a_nat[:, :], ac[bb * 128:(bb + 1) * 128, :])
            transpose_to_sbuf(saT[2 * HID:2 * HID + A_DIM, bb * 128:(bb + 1) * 128],
                              a_nat[:, :])

        # MLP head weights, transposed
        head_sb = []
        for (w1, b1, w2, b2, w3, b3) in heads:
            w1_nat = scratch.tile([128, 128], F32, tag="w1nat")
            w1T = P.tile([128, 256], F32, tag=f"w1T{len(head_sb)}")
            for rh in range(2):
                nc.sync.dma_start(w1_nat[:, :], w1[rh * 128:(rh + 1) * 128, :])
                transpose_to_sbuf(w1T[:, rh * 128:(rh + 1) * 128], w1_nat[:, :])
            w2T = [P.tile([128, 256], F32, tag=f"w2T{len(head_sb)}_{kh}",
                          name=f"w2T{len(head_sb)}_{kh}")
                   for kh in range(2)]
            for rh in range(2):
                for kh in range(2):
                    w2_nat = scratch.tile([128, 128], F32, tag="w2nat")
                    nc.sync.dma_start(
                        w2_nat[:, :],
                        w2[rh * 128:(rh + 1) * 128, kh * 128:(kh + 1) * 128])
                    transpose_to_sbuf(w2T[kh][:, rh * 128:(rh + 1) * 128],
                                      w2_nat[:, :])
            w3T = P.tile([128, 2], F32, tag=f"w3T{len(head_sb)}")
            for kh in range(2):
                nc.sync.dma_start(w3T[:, kh:kh + 1],
                                  w3[0, kh * 128:(kh + 1) * 128][:, None])
            b1c = P.tile([128, 2], F32, tag=f"b1c{len(head_sb)}")
            b2c = P.tile([128, 2], F32, tag=f"b2c{len(head_sb)}")
            for rh in range(2):
                nc.sync.dma_start(b1c[:, rh:rh + 1],
                                  b1[rh * 128:(rh + 1) * 128][:, None])
                nc.sync.dma_start(b2c[:, rh:rh + 1],
                                  b2[rh * 128:(rh + 1) * 128][:, None])
            b3c = P.tile([1, 1], F32, tag=f"b3c{len(head_sb)}")
            nc.sync.dma_start(b3c[:, :], b3[:][None, :])
            head_sb.append((w1T, w2T, w3T, b1c, b2c, b3c))

        ctxA.close()

        # ---------------- Phase B: main token stream ----------------
        ctxB = ctx.enter_context(ExitStack())
        xpool = ctx.enter_context(tc.tile_pool(name="xchunk", bufs=3))
        ppool = ctx.enter_context(tc.tile_pool(name="prod", bufs=2))
        tpool = ctx.enter_context(tc.tile_pool(name="tbuf", bufs=2))
        spool = ctx.enter_context(tc.tile_pool(name="sall", bufs=2))
        jpool = ctx.enter_context(tc.tile_pool(name="junk", bufs=2))
        ps_m = ctxB.enter_context(tc.tile_pool(name="ps_m", bufs=2, space="PSUM"))

        m_nat = [P.tile([128, XW], F32, tag=f"mnat{b}", name=f"mnat{b}")
                 for b in range(nblk)]
        mT = P.tile([L_DIM, b_loc], F32, tag="mT")
        s_row = P.tile([1, b_loc], F32, tag="srow")

        for ch in range(nchunk):
            x_ch = xpool.tile([128, PERIOD * XW], F32, tag="xch")
            x3 = x_ch[:, :].rearrange("p (j d) -> p j d", d=XW)
            src = ls[ch * PERIOD * 128:(ch + 1) * PERIOD * 128, :]
            nc.sync.dma_start(x3[:, :, 0:L_DIM],
                              src.rearrange("(p t) d -> p t d", t=PERIOD))
            if ch < 3:
                # ones column; x pool cycles 3 buffers, col 128 is never
                # overwritten so 3 writes cover the whole run
                nc.vector.memset(x3[:, :, 128:129], 1.0)

            prod = ppool.tile([128, PERIOD * 128], F32, tag="pr")
            p3 = prod[:, :].rearrange("p (j d) -> p j d", d=128)
            nc.gpsimd.tensor_tensor(
                out=p3[:, 0:G_TILES, :],
                in0=x3[:, 0:G_TILES, 0:128],
                in1=v_rep[:, None, :].broadcast_to((128, G_TILES, 128)),
                op=AF.mult)
            nc.vector.tensor_tensor(
                out=p3[:, G_TILES:PERIOD, :],
                in0=x3[:, G_TILES:PERIOD, 0:128],
                in1=v_rep[:, None, :].broadcast_to((128, PERIOD - G_TILES, 128)),
                op=AF.mult)

            t_buf = tpool.tile([128, PERIOD], F32, tag="tb")
            nc.vector.tensor_reduce(out=t_buf[:, 0:R_TILES],
                                    in_=p3[:, 0:R_TILES, :],
                                    axis=mybir.AxisListType.X, op=AF.add)
            junk = jpool.tile([128, 128], F32, tag="jk")
            for j in range(R_TILES, PERIOD):
                nc.scalar.activation(junk[:, :], p3[:, j, :], ACTF.Identity,
                                     accum_out=t_buf[:, j:j + 1])

            score = tpool.tile([128, PERIOD], F32, tag="sc")
            nc.scalar.activation(score[:, :], t_buf[:, :], ACTF.Lrelu,
                                 bias=c_all[:, ch:ch + 1], alpha=0.01)

            s_all = spool.tile([128, PERIOD * SAMP], F32, tag="sa")
            sa3 = s_all[:, :].rearrange("p (j g) -> p j g", g=SAMP)
            nc.vector.tensor_tensor(
                out=sa3[:, :, :],
                in0=score[:, :, None].broadcast_to((128, PERIOD, SAMP)),
                in1=gb_sb[:, None, :].broadcast_to((128, PERIOD, SAMP)),
                op=AF.mult)

            m_ps = ps_m.tile([SAMP, 129], F32, tag="mps")
            for j in range(PERIOD):
                nc.tensor.matmul(out=m_ps[:, :], lhsT=sa3[:, j, :],
                                 rhs=x3[:, j, 0:129],
                                 start=(j == 0), stop=(j == PERIOD - 1))
            # engines can't write at 16-partition offsets; stage at partition 0
            # and let DMA do the partition shift
            m_stage = jpool.tile([SAMP, 132], F32, tag="mstg")
            nc.scalar.copy(m_stage[:, 0:129], m_ps[:, :])
            blk, row = ch // 8, (ch % 8) * SAMP
            nc.sync.dma_start(m_nat[blk][row:row + SAMP, 0:129],
                              m_stage[:, 0:129])

        # m_nat [sample, feat] -> mT [feat, sample]; s column -> s_row
        ps_f = ctxB.enter_context(tc.tile_pool(name="ps_f", bufs=2, space="PSUM"))
        s_nat = P.tile([128, nblk], F32, tag="snat")
        for b in range(nblk):
            f_ps = ps_f.tile([128, 128], F32, tag="fps")
            nc.tensor.transpose(f_ps[:, :], m_nat[b][:, 0:128], ident[:, :])
            nc.scalar.copy(mT[:, b * 128:(b + 1) * 128], f_ps[:, :])
            nc.vector.tensor_copy(s_nat[:, b:b + 1], m_nat[b][:, 128:129])
        sn_ps = ps_f.tile([nblk, 128], F32, tag="fps")
        nc.tensor.transpose(sn_ps[:, :], s_nat[:, :], ident[:, :])
        s_nat4 = P.tile([nblk, 128], F32, tag="snat4")
        nc.scalar.copy(s_nat4[:, :], sn_ps[:, :])
        for b in range(nblk):
            nc.sync.dma_start(s_row[0:1, b * 128:(b + 1) * 128],
                              s_nat4[b:b + 1, :])

        ctxB.close()

        # ---------------- Phase C: combine + heads ----------------
        ps_c = ctx.enter_context(tc.tile_pool(name="ps_c", bufs=4, space="PSUM"))

        total = P.tile([1, b_loc], F32, tag="total")
        nc.vector.tensor_tensor(out=total[:, :], in0=sg_raw[:, :], in1=s_row[:, :],
                                op=AF.add)
        recip = P.tile([1, b_loc], F32, tag="recip")
        nc.vector.reciprocal(recip[:, :], total[:, :])
        gn_row = P.tile([1, b_loc], F32, tag="gn")
        nc.vector.tensor_tensor(out=gn_row[:, :], in0=sg_raw[:, :], in1=recip[:, :],
                                op=AF.mult)

        r32_ps = ps_c.tile([HID, b_loc], F32, tag="cps")
        nc.tensor.matmul(out=r32_ps[:, :], lhsT=ones_row[0:1, 0:HID], rhs=recip[:, :])
        r32 = P.tile([HID, b_loc], F32, tag="r32")
        nc.scalar.copy(r32[:, :], r32_ps[:, :])
        g32_ps = ps_c.tile([HID, b_loc], F32, tag="cps")
        nc.tensor.matmul(out=g32_ps[:, :], lhsT=ones_row[0:1, 0:HID], rhs=gn_row[:, :])
        g32 = P.tile([HID, b_loc], F32, tag="g32")
        nc.scalar.copy(g32[:, :], g32_ps[:, :])

        lT_ps = ps_c.tile([HID, b_loc], F32, tag="cps")
        nc.tensor.matmul(out=lT_ps[:, :], lhsT=UwT[:, :], rhs=mT[:, :],
                         start=True, stop=False)
        nc.tensor.matmul(out=lT_ps[:, :], lhsT=Ub_row[:, :], rhs=s_row[:, :],
                         start=False, stop=True)

        lnorm = P.tile([HID, b_loc], F32, tag="lnorm")
        nc.vector.tensor_tensor(out=lnorm[:, :], in0=lT_ps[:, :], in1=r32[:, :],
                                op=AF.mult)
        gpart = P.tile([HID, b_loc], F32, tag="gpart")
        nc.vector.tensor_tensor(out=gpart[:, :], in0=wgT[:, :], in1=g32[:, :],
                                op=AF.mult)
        nc.scalar.activation(saT[0:HID, :], gpart[:, :], ACTF.Relu)
        nc.scalar.activation(saT[HID:2 * HID, :], lnorm[:, :], ACTF.Relu)

        for h, (w1T, w2T, w3T, b1c, b2c, b3c) in enumerate(head_sb):
            h1 = []
            for rh in range(2):
                h_ps = ps_c.tile([128, b_loc], F32, tag="cps")
                nc.tensor.matmul(out=h_ps[:, :], lhsT=w1T[:, rh * 128:(rh + 1) * 128],
                                 rhs=saT[:, :])
                h_sb = scratch.tile([128, b_loc], F32, tag="h1sb")
                nc.scalar.activation(h_sb[:, :], h_ps[:, :], ACTF.Relu,
                                     bias=b1c[:, rh:rh + 1])
                h1.append(h_sb)
            h2 = []
            for rh in range(2):
                h_ps = ps_c.tile([128, b_loc], F32, tag="cps")
                for kh in range(2):
                    nc.tensor.matmul(out=h_ps[:, :],
                                     lhsT=w2T[kh][:, rh * 128:(rh + 1) * 128],
                                     rhs=h1[kh][:, :],
                                     start=(kh == 0), stop=(kh == 1))
                h_sb = scratch.tile([128, b_loc], F32, tag="h2sb")
                nc.scalar.activation(h_sb[:, :], h_ps[:, :], ACTF.Relu,
                                     bias=b2c[:, rh:rh + 1])
                h2.append(h_sb)
            q_ps = ps_c.tile([1, b_loc], F32, tag="cps")
            for kh in range(2):
                nc.tensor.matmul(out=q_ps[:, :], lhsT=w3T[:, kh:kh + 1],
                                 rhs=h2[kh][:, :], start=(kh == 0), stop=(kh == 1))
            q_row = scratch.tile([1, b_loc], F32, tag="qrow")
            nc.scalar.activation(q_row[:, :], q_ps[:, :], ACTF.Identity,
                                 bias=b3c[:, :])
            nc.sync.dma_start(out_d[h:h + 1, :], q_row[:, :])

    nc.compile()
    return nc


def _shard_inputs(inputs, b_loc=B_LOC):
    """Full inputs -> list of per-core in_maps."""
    gb = _make_gb()
    perm8 = _make_perm8()
    maps = []
    for c in range(NCORES):
        sl = slice(c * b_loc, (c + 1) * b_loc)
        m = {
            "local_states": np.ascontiguousarray(
                inputs["local_states"][sl].reshape(b_loc * L, L_DIM)),
            "global_states": np.ascontiguousarray(inputs["global_states"][sl]),
            "actions": np.ascontiguousarray(inputs["actions"][sl]),
            "gb": gb,
            "perm8": perm8,
        }
        for k in ("W_w", "W_b", "U_w", "U_b", "att_b",
                  "l1_w", "l1_b", "l2_w", "l2_b", "l3_w", "l3_b",
                  "l4_w", "l4_b", "l5_w", "l5_b", "l6_w", "l6_b"):
            m[k] = np.ascontiguousarray(np.asarray(inputs[k], np.float32))
        m["att_w"] = np.ascontiguousarray(
            np.asarray(inputs["att_w"], np.float32).reshape(1, 2 * HID))
        maps.append(m)
    return maps


_CACHE = {}


def kernel(**inputs) -> np.ndarray:
    from concourse.bass_utils import run_bass_kernel_spmd

    inputs = {k: np.asarray(v, np.float32) for k, v in inputs.items()}
    if "nc" not in _CACHE:
        _CACHE["nc"] = build_bass()
    nc = _CACHE["nc"]
    maps = _shard_inputs(inputs)
    res = run_bass_kernel_spmd(nc, maps, list(range(NCORES)))
    outs = [res.results[c]["out"] for c in range(NCORES)]  # each [2, B_LOC]
    q = np.concatenate(outs, axis=1)  # [2, B]
    return q.reshape(2, B, 1).astype(np.float32)


# revision 9
# speedup vs baseline: 1.4084x; 1.0650x over previous
"""Trainium2 Bass kernel for the Critic (gnn_message_passing) problem.

Math (per sample b):
  wg   = W_w @ g + W_b                                  [32]
  score_l = lrelu(x_l . v + c_b),  v = U_w^T a2,  c_b = a1.wg + att_b + U_b.a2
  score_g = lrelu((a1+a2).wg + att_b)
  total = score_g + sum_l score_l
  l_part = (U_w @ m_b + U_b * s_b) / total   with m_b = sum_l score_l x_l, s_b = sum_l score_l
  g_part = (score_g / total) * wg
  sa = [relu(g_part); relu(l_part); action]            [128]
  q_h = l3 @ relu(l2 @ relu(l1 @ sa + b1) + b2) + b3   (two heads)

Layout strategy (one NeuronCore handles B_LOC=512 samples, pure data parallel x8):
  Tokens are laid out BLOCKED: within a 16-sample chunk (3200 tokens), SBUF
  partition p holds tokens [25p, 25p+25).  Since 200 = 8*25, partition p
  always belongs to sample p//8 - no sample straddles a partition, so the
  attention bias c is a per-partition scalar and no boundary masking exists.
  DMA moves 12.8KB contiguous per partition per chunk.

  Per chunk (25 column-tiles of [128, 128]):
   - mult x*v: one big GPSIMD op (tiles 0:G) + one big DVE op (tiles G:25)
   - reduce to t[128,25]: one segmented DVE tensor_reduce (tiles 0:R) +
     per-tile ACT accum reduces (tiles R:25)
   - score = Lrelu(t + c) in one ACT op (c per-partition bias)
   - S_all[p,t',g] = score[p,t'] * [p//8==g]: one DVE mult with the
     block mask; lhsT of the m-matmul.
   - m^T[16,129] accumulated in PSUM over 25 matmuls: lhsT=S_all[:,t',:]
     (16 cols, cheap LDWEIGHTS), rhs=[x_tile | ones] (s_b rides along as
     column 128).
  At the end m (sample-major) is PE-transposed to feature-major mT for the
  combine; heads run feature-major on PE as in the reference mapping.
"""
import os
import sys

sys.path.insert(0, "/opt/trn_rl_repo")

from contextlib import ExitStack

import numpy as np

import concourse.bass as bass
import concourse.tile as tile
from concourse import bacc
from concourse import mybir

F32 = mybir.dt.float32
AF = mybir.AluOpType
ACTF = mybir.ActivationFunctionType

G_DIM, L_DIM, A_DIM, HID = 256, 128, 64, 32
B, L = 4096, 200
NCORES = 8
B_LOC = B // NCORES          # 512 samples per core
PERIOD = 25                  # column-tiles per chunk; 16 samples per chunk
SAMP = 16                    # samples per chunk
XW = 132                     # x_ch row stride in elems (129 used, pad 3)
G_TILES = 15                 # tiles whose x*v mult runs on GPSIMD
R_TILES = 20                 # tiles reduced by the segmented DVE reduce


def _make_gb():
    gb = np.zeros((128, SAMP), np.float32)
    for p in range(128):
        gb[p, p // 8] = 1.0
    return gb


def _make_perm8():
    # perm[q, 128k+p] = 1 iff q == 16k + p//8; used as lhsT so that
    # (perm_k)^T @ c_colblk gives c_all[p] = c[16k + p//8]
    m = np.zeros((128, 8 * 128), np.float32)
    for k in range(8):
        for p in range(128):
            m[16 * k + p // 8, 128 * k + p] = 1.0
    return m


def build_bass(b_loc=B_LOC):
    tok = b_loc * L
    nchunk = tok // (PERIOD * 128)     # 32
    nblk = nchunk // 8                 # 4 sample-blocks of 128

    nc = bacc.Bacc()

    ls = nc.dram_tensor("local_states", [tok, L_DIM], F32, kind="ExternalInput")
    gs = nc.dram_tensor("global_states", [b_loc, G_DIM], F32, kind="ExternalInput")
    ac = nc.dram_tensor("actions", [b_loc, A_DIM], F32, kind="ExternalInput")
    Ww = nc.dram_tensor("W_w", [HID, G_DIM], F32, kind="ExternalInput")
    Wb = nc.dram_tensor("W_b", [HID], F32, kind="ExternalInput")
    Uw = nc.dram_tensor("U_w", [HID, L_DIM], F32, kind="ExternalInput")
    Ub = nc.dram_tensor("U_b", [HID], F32, kind="ExternalInput")
    attw = nc.dram_tensor("att_w", [1, 2 * HID], F32, kind="ExternalInput")
    attb = nc.dram_tensor("att_b", [1], F32, kind="ExternalInput")
    heads = []
    for h, names in enumerate((("l1", "l2", "l3"), ("l4", "l5", "l6"))):
        w1 = nc.dram_tensor(f"{names[0]}_w", [256, 128], F32, kind="ExternalInput")
        b1 = nc.dram_tensor(f"{names[0]}_b", [256], F32, kind="ExternalInput")
        w2 = nc.dram_tensor(f"{names[1]}_w", [256, 256], F32, kind="ExternalInput")
        b2 = nc.dram_tensor(f"{names[1]}_b", [256], F32, kind="ExternalInput")
        w3 = nc.dram_tensor(f"{names[2]}_w", [1, 256], F32, kind="ExternalInput")
        b3 = nc.dram_tensor(f"{names[2]}_b", [1], F32, kind="ExternalInput")
        heads.append((w1, b1, w2, b2, w3, b3))
    gb_d = nc.dram_tensor("gb", [128, SAMP], F32, kind="ExternalInput")
    perm_d = nc.dram_tensor("perm8", [128, 8 * 128], F32, kind="ExternalInput")
    out_d = nc.dram_tensor("out", [2, b_loc], F32, kind="ExternalOutput")

    nb = b_loc // 128

    with tile.TileContext(nc) as tc, ExitStack() as ctx:
        P = ctx.enter_context(tc.tile_pool(name="persist", bufs=1))
        scratch = ctx.enter_context(tc.tile_pool(name="scratch", bufs=2))
        ctxA = ctx.enter_context(ExitStack())
        ps_t = ctxA.enter_context(tc.tile_pool(name="ps_t", bufs=2, space="PSUM"))

        # ---------------- Phase A: constants & small precompute ----------------
        from concourse.masks import make_identity

        ident = P.tile([128, 128], F32, tag="ident")
        make_identity(nc, ident[:, :])
        ones_row = P.tile([1, 128], F32, tag="onesr")
        nc.vector.memset(ones_row[:, :], 1.0)
        gb_sb = P.tile([128, SAMP], F32, tag="gb")
        nc.sync.dma_start(gb_sb[:, :], gb_d[:, :])
        perm_sb = P.tile([128, 8 * 128], F32, tag="perm")
        nc.sync.dma_start(perm_sb[:, :], perm_d[:, :])

        def transpose_to_sbuf(dst_ap, src_ap):
            """dst[f, p] = src[p, f] via PE transpose + ACT copy out of PSUM."""
            pp, ff = src_ap.shape
            t_ps = ps_t.tile([128, 128], F32, tag="tps")
            nc.tensor.transpose(t_ps[0:ff, 0:pp], src_ap, ident[0:pp, 0:pp])
            nc.scalar.copy(dst_ap, t_ps[0:ff, 0:pp])

        # small weights
        Ww_sb = P.tile([HID, G_DIM], F32, tag="Ww")
        nc.sync.dma_start(Ww_sb[:, :], Ww[:, :])
        Wb_sb = P.tile([HID, 1], F32, tag="Wb")
        nc.sync.dma_start(Wb_sb[:, :], Wb[:][:, None])
        Uw_sb = P.tile([HID, L_DIM], F32, tag="Uw")
        nc.sync.dma_start(Uw_sb[:, :], Uw[:, :])
        Ub_col = P.tile([HID, 1], F32, tag="Ubc")
        nc.sync.dma_start(Ub_col[:, :], Ub[:][:, None])
        Ub_row = P.tile([1, HID], F32, tag="Ubr")
        nc.sync.dma_start(Ub_row[:, :], Ub[:][None, :])
        a1_sb = P.tile([HID, 1], F32, tag="a1")
        nc.sync.dma_start(a1_sb[:, :], attw[0, 0:HID][:, None])
        a2_sb = P.tile([HID, 1], F32, tag="a2")
        nc.sync.dma_start(a2_sb[:, :], attw[0, HID:2 * HID][:, None])
        attb_sb = P.tile([1, 1], F32, tag="attb")
        nc.sync.dma_start(attb_sb[:, :], attb[:][None, :])

        WwT = []  # W_w^T in [128, HID] chunks over G_DIM
        for g in range(G_DIM // 128):
            w = P.tile([128, HID], F32, tag=f"WwT{g}")
            transpose_to_sbuf(w[:, :], Ww_sb[:, g * 128:(g + 1) * 128])
            WwT.append(w)
        UwT = P.tile([L_DIM, HID], F32, tag="UwT")
        transpose_to_sbuf(UwT[:, :], Uw_sb[:, :])

        # gT: global_states^T  [G_DIM partition-chunks][128, b_loc]
        gT = []
        for g in range(G_DIM // 128):
            t = P.tile([128, b_loc], F32, tag=f"gT{g}")
            gT.append(t)
        for bb in range(nb):
            g_nat = scratch.tile([128, G_DIM], F32, tag="gnat")
            nc.sync.dma_start(g_nat[:, :], gs[bb * 128:(bb + 1) * 128, :])
            for g in range(G_DIM // 128):
                transpose_to_sbuf(gT[g][:, bb * 128:(bb + 1) * 128],
                                  g_nat[:, g * 128:(g + 1) * 128])

        # wg^T [HID, b_loc]
        wgT_ps = ps_t.tile([HID, b_loc], F32, tag="wps")
        for g in range(G_DIM // 128):
            nc.tensor.matmul(out=wgT_ps[:, :], lhsT=WwT[g][:, :], rhs=gT[g][:, :],
                             start=(g == 0), stop=(g == G_DIM // 128 - 1))
        wgT = P.tile([HID, b_loc], F32, tag="wgT")
        nc.scalar.activation(wgT[:, :], wgT_ps[:, :], ACTF.Identity, bias=Wb_sb[:, :])

        # v_row [1, 128] = a2^T U_w ;  v_rep [128, 128] = ones (x) v_row
        v_ps = ps_t.tile([1, L_DIM], F32, tag="tps")
        nc.tensor.matmul(out=v_ps[:, :], lhsT=a2_sb[:, :], rhs=Uw_sb[:, :])
        v_row = P.tile([1, L_DIM], F32, tag="vrow")
        nc.scalar.copy(v_row[:, :], v_ps[:, :])
        vrep_ps = ps_t.tile([128, 128], F32, tag="tps")
        nc.tensor.matmul(out=vrep_ps[:, :], lhsT=ones_row[:, :], rhs=v_row[:, :])
        v_rep = P.tile([128, 128], F32, tag="vrep")
        nc.scalar.copy(v_rep[:, :], vrep_ps[:, :])

        # cconst = Ub.a2 + att_b, replicated to [128, 1]
        uba2_ps = ps_t.tile([1, 1], F32, tag="tps")
        nc.tensor.matmul(out=uba2_ps[:, :], lhsT=Ub_col[:, :], rhs=a2_sb[:, :])
        cconst = P.tile([1, 1], F32, tag="cconst")
        nc.vector.tensor_tensor(out=cconst[:, :], in0=uba2_ps[:, :], in1=attb_sb[:, :],
                                op=AF.add)
        ccol_ps = ps_t.tile([128, 1], F32, tag="tps")
        nc.tensor.matmul(out=ccol_ps[:, :], lhsT=ones_row[:, :], rhs=cconst[:, :])
        cconst_col = P.tile([128, 1], F32, tag="cconstc")
        nc.scalar.copy(cconst_col[:, :], ccol_ps[:, :])

        # c_colblk [128, nblk]: c_lin of sample 128b+q at (q, b) (no const yet)
        c_colblk = P.tile([128, nblk], F32, tag="ccb")
        for b in range(nblk):
            cb_ps = ps_t.tile([128, 1], F32, tag="tps")
            nc.tensor.matmul(out=cb_ps[:, :], lhsT=wgT[:, b * 128:(b + 1) * 128],
                             rhs=a1_sb[:, :])
            nc.scalar.copy(c_colblk[:, b:b + 1], cb_ps[:, :])

        # c_all [128, nchunk]: c (incl const) for partition p in chunk ch
        call_ps = ps_t.tile([128, nchunk], F32, tag="cps2")
        for k in range(8):
            nc.tensor.matmul(out=call_ps[:, k:nchunk:8],
                             lhsT=perm_sb[:, k * 128:(k + 1) * 128],
                             rhs=c_colblk[:, :], start=True, stop=True,
                             skip_group_check=True)
        c_all = P.tile([128, nchunk], F32, tag="call")
        nc.scalar.activation(c_all[:, :], call_ps[:, :], ACTF.Identity,
                             bias=cconst_col[:, :])

        # sg_raw [1, b_loc] = lrelu((a1+a2).wg + att_b)
        a12 = P.tile([HID, 1], F32, tag="a12")
        nc.vector.tensor_tensor(out=a12[:, :], in0=a1_sb[:, :], in1=a2_sb[:, :],
                                op=AF.add)
        sg_ps = ps_t.tile([1, b_loc], F32, tag="tps")
        nc.tensor.matmul(out=sg_ps[:, :], lhsT=a12[:, :], rhs=wgT[:, :])
        sg_raw = P.tile([1, b_loc], F32, tag="sg_raw")
        nc.scalar.activation(sg_raw[:, :], sg_ps[:, :], ACTF.Lrelu,
                             bias=attb_sb[:, :], alpha=0.01)

        # actions^T into sa^T[64:128]
        saT = P.tile([128, b_loc], F32, tag="saT")
        for bb in range(nb):
            a_nat = scratch.tile([128, A_DIM], F32, tag="anat")
            nc.sync.dma_start(a_nat[:, :], ac[bb * 128:(bb + 1) * 128, :])
            transpose_to_sbuf(saT[2 * HID:2 * HID + A_DIM, bb * 128:(bb + 1) * 128],
                              a_nat[:, :])

        # MLP head weights, transposed
        head_sb = []
        for (w1, b1, w2, b2, w3, b3) in heads:
            w1_nat = scratch.tile([128, 128], F32, tag="w1nat")
            w1T = P.tile([128, 256], F32, tag=f"w1T{len(head_sb)}")
            for rh in range(2):
                nc.sync.dma_start(w1_nat[:, :], w1[rh * 128:(rh + 1) * 128, :])
                transpose_to_sbuf(w1T[:, rh * 128:(rh + 1) * 128], w1_nat[:, :])
            w2T = [P.tile([128, 256], F32, tag=f"w2T{len(head_sb)}_{kh}",
                          name=f"w2T{len(head_sb)}_{kh}")
                   for kh in range(2)]
            for rh in range(2):
                for kh in range(2):
                    w2_nat = scratch.tile([128, 128], F32, tag="w2nat")
                    nc.sync.dma_start(
                        w2_nat[:, :],
                        w2[rh * 128:(rh + 1) * 128, kh * 128:(kh + 1) * 128])
                    transpose_to_sbuf(w2T[kh][:, rh * 128:(rh + 1) * 128],
                                      w2_nat[:, :])
            w3T = P.tile([128, 2], F32, tag=f"w3T{len(head_sb)}")
            for kh in range(2):
                nc.sync.dma_start(w3T[:, kh:kh + 1],
                                  w3[0, kh * 128:(kh + 1) * 128][:, None])
            b1c = P.tile([128, 2], F32, tag=f"b1c{len(head_sb)}")
            b2c = P.tile([128, 2], F32, tag=f"b2c{len(head_sb)}")
            for rh in range(2):
                nc.sync.dma_start(b1c[:, rh:rh + 1],
                                  b1[rh * 128:(rh + 1) * 128][:, None])
                nc.sync.dma_start(b2c[:, rh:rh + 1],
                                  b2[rh * 128:(rh + 1) * 128][:, None])
            b3c = P.tile([1, 1], F32, tag=f"b3c{len(head_sb)}")
            nc.sync.dma_start(b3c[:, :], b3[:][None, :])
            head_sb.append((w1T, w2T, w3T, b1c, b2c, b3c))

        ctxA.close()

        # ---------------- Phase B: main token stream ----------------
        ctxB = ctx.enter_context(ExitStack())
        xpool = ctx.enter_context(tc.tile_pool(name="xchunk", bufs=3))
        ppool = ctx.enter_context(tc.tile_pool(name="prod", bufs=2))
        tpool = ctx.enter_context(tc.tile_pool(name="tbuf", bufs=2))
        spool = ctx.enter_context(tc.tile_pool(name="sall", bufs=2))
        jpool = ctx.enter_context(tc.tile_pool(name="junk", bufs=2))
        ps_m = ctxB.enter_context(tc.tile_pool(name="ps_m", bufs=2, space="PSUM"))

        m_nat = [P.tile([128, XW], F32, tag=f"mnat{b}", name=f"mnat{b}")
                 for b in range(nblk)]
        mT = P.tile([L_DIM, b_loc], F32, tag="mT")
        s_row = P.tile([1, b_loc], F32, tag="srow")

        # software-pipelined emission: stage A (DMA + mults) of chunk c+1 is
        # emitted before stage B (reduce..matmuls) of chunk c so in-order
        # engines always have next-chunk work queued ahead of cross-engine
        # waits; the PSUM copy-out lags one more chunk.
        live = {}

        def emit_A(ch):
            x_ch = xpool.tile([128, PERIOD * XW], F32, tag="xch")
            x3 = x_ch[:, :].rearrange("p (j d) -> p j d", d=XW)
            src = ls[ch * PERIOD * 128:(ch + 1) * PERIOD * 128, :]
            nc.sync.dma_start(x3[:, :, 0:L_DIM],
                              src.rearrange("(p t) d -> p t d", t=PERIOD))
            if ch < 3:
                # ones column; x pool cycles 3 buffers, col 128 is never
                # overwritten so 3 writes cover the whole run
                nc.vector.memset(x3[:, :, 128:129], 1.0)

            prod = ppool.tile([128, PERIOD * 128], F32, tag="pr")
            p3 = prod[:, :].rearrange("p (j d) -> p j d", d=128)
            nc.gpsimd.tensor_tensor(
                out=p3[:, 0:G_TILES, :],
                in0=x3[:, 0:G_TILES, 0:128],
                in1=v_rep[:, None, :].broadcast_to((128, G_TILES, 128)),
                op=AF.mult)
            nc.vector.tensor_tensor(
                out=p3[:, G_TILES:PERIOD, :],
                in0=x3[:, G_TILES:PERIOD, 0:128],
                in1=v_rep[:, None, :].broadcast_to((128, PERIOD - G_TILES, 128)),
                op=AF.mult)
            live[ch] = {"x3": x3, "p3": p3}

        def emit_B(ch):
            x3, p3 = live[ch]["x3"], live[ch]["p3"]
            t_buf = tpool.tile([128, PERIOD], F32, tag="tb")
            nc.vector.tensor_reduce(out=t_buf[:, 0:R_TILES],
                                    in_=p3[:, 0:R_TILES, :],
                                    axis=mybir.AxisListType.X, op=AF.add)
            junk = jpool.tile([128, 128], F32, tag="jk")
            for j in range(R_TILES, PERIOD):
                nc.scalar.activation(junk[:, :], p3[:, j, :], ACTF.Identity,
                                     accum_out=t_buf[:, j:j + 1])

            score = tpool.tile([128, PERIOD], F32, tag="sc")
            nc.scalar.activation(score[:, :], t_buf[:, :], ACTF.Lrelu,
                                 bias=c_all[:, ch:ch + 1], alpha=0.01)

            s_all = spool.tile([128, PERIOD * SAMP], F32, tag="sa")
            sa3 = s_all[:, :].rearrange("p (j g) -> p j g", g=SAMP)
            nc.vector.tensor_tensor(
                out=sa3[:, :, :],
                in0=score[:, :, None].broadcast_to((128, PERIOD, SAMP)),
                in1=gb_sb[:, None, :].broadcast_to((128, PERIOD, SAMP)),
                op=AF.mult)

            m_ps = ps_m.tile([SAMP, 129], F32, tag="mps")
            for j in range(PERIOD):
                nc.tensor.matmul(out=m_ps[:, :], lhsT=sa3[:, j, :],
                                 rhs=x3[:, j, 0:129],
                                 start=(j == 0), stop=(j == PERIOD - 1))
            live[ch]["m_ps"] = m_ps

        def emit_copy(ch):
            # engines can't write at 16-partition offsets; stage at partition 0
            # and let DMA do the partition shift
            m_ps = live[ch].pop("m_ps")
            m_stage = jpool.tile([SAMP, 132], F32, tag="mstg")
            nc.scalar.copy(m_stage[:, 0:129], m_ps[:, :])
            blk, row = ch // 8, (ch % 8) * SAMP
            nc.sync.dma_start(m_nat[blk][row:row + SAMP, 0:129],
                              m_stage[:, 0:129])
            del live[ch]

        emit_A(0)
        for ch in range(1, nchunk):
            emit_A(ch)
            emit_B(ch - 1)
            if ch >= 2:
                emit_copy(ch - 2)
        emit_B(nchunk - 1)
        emit_copy(nchunk - 2)
        emit_copy(nchunk - 1)

        # m_nat [sample, feat] -> mT [feat, sample]; s column -> s_row
        ps_f = ctxB.enter_context(tc.tile_pool(name="ps_f", bufs=2, space="PSUM"))
        s_nat = P.tile([128, nblk], F32, tag="snat")
        for b in range(nblk):
            f_ps = ps_f.tile([128, 128], F32, tag="fps")
            nc.tensor.transpose(f_ps[:, :], m_nat[b][:, 0:128], ident[:, :])
            nc.scalar.copy(mT[:, b * 128:(b + 1) * 128], f_ps[:, :])
            nc.vector.tensor_copy(s_nat[:, b:b + 1], m_nat[b][:, 128:129])
        sn_ps = ps_f.tile([nblk, 128], F32, tag="fps")
        nc.tensor.transpose(sn_ps[:, :], s_nat[:, :], ident[:, :])
        s_nat4 = P.tile([nblk, 128], F32, tag="snat4")
        nc.scalar.copy(s_nat4[:, :], sn_ps[:, :])
        for b in range(nblk):
            nc.sync.dma_start(s_row[0:1, b * 128:(b + 1) * 128],
                              s_nat4[b:b + 1, :])

        ctxB.close()

        # ---------------- Phase C: combine + heads ----------------
        ps_c = ctx.enter_context(tc.tile_pool(name="ps_c", bufs=4, space="PSUM"))

        total = P.tile([1, b_loc], F32, tag="total")
        nc.vector.tensor_tensor(out=total[:, :], in0=sg_raw[:, :], in1=s_row[:, :],
                                op=AF.add)
        recip = P.tile([1, b_loc], F32, tag="recip")
        nc.vector.reciprocal(recip[:, :], total[:, :])
        gn_row = P.tile([1, b_loc], F32, tag="gn")
        nc.vector.tensor_tensor(out=gn_row[:, :], in0=sg_raw[:, :], in1=recip[:, :],
                                op=AF.mult)

        r32_ps = ps_c.tile([HID, b_loc], F32, tag="cps")
        nc.tensor.matmul(out=r32_ps[:, :], lhsT=ones_row[0:1, 0:HID], rhs=recip[:, :])
        r32 = P.tile([HID, b_loc], F32, tag="r32")
        nc.scalar.copy(r32[:, :], r32_ps[:, :])
        g32_ps = ps_c.tile([HID, b_loc], F32, tag="cps")
        nc.tensor.matmul(out=g32_ps[:, :], lhsT=ones_row[0:1, 0:HID], rhs=gn_row[:, :])
        g32 = P.tile([HID, b_loc], F32, tag="g32")
        nc.scalar.copy(g32[:, :], g32_ps[:, :])

        lT_ps = ps_c.tile([HID, b_loc], F32, tag="cps")
        nc.tensor.matmul(out=lT_ps[:, :], lhsT=UwT[:, :], rhs=mT[:, :],
                         start=True, stop=False)
        nc.tensor.matmul(out=lT_ps[:, :], lhsT=Ub_row[:, :], rhs=s_row[:, :],
                         start=False, stop=True)

        lnorm = P.tile([HID, b_loc], F32, tag="lnorm")
        nc.vector.tensor_tensor(out=lnorm[:, :], in0=lT_ps[:, :], in1=r32[:, :],
                                op=AF.mult)
        gpart = P.tile([HID, b_loc], F32, tag="gpart")
        nc.vector.tensor_tensor(out=gpart[:, :], in0=wgT[:, :], in1=g32[:, :],
                                op=AF.mult)
        nc.scalar.activation(saT[0:HID, :], gpart[:, :], ACTF.Relu)
        nc.scalar.activation(saT[HID:2 * HID, :], lnorm[:, :], ACTF.Relu)

        for h, (w1T, w2T, w3T, b1c, b2c, b3c) in enumerate(head_sb):
            h1 = []
            for rh in range(2):
                h_ps = ps_c.tile([128, b_loc], F32, tag="cps")
                nc.tensor.matmul(out=h_ps[:, :], lhsT=w1T[:, rh * 128:(rh + 1) * 128],
                                 rhs=saT[:, :])
                h_sb = scratch.tile([128, b_loc], F32, tag="h1sb")
                nc.scalar.activation(h_sb[:, :], h_ps[:, :], ACTF.Relu,
                                     bias=b1c[:, rh:rh + 1])
                h1.append(h_sb)
            h2 = []
            for rh in range(2):
                h_ps = ps_c.tile([128, b_loc], F32, tag="cps")
                for kh in range(2):
                    nc.tensor.matmul(out=h_ps[:, :],
                                     lhsT=w2T[kh][:, rh * 128:(rh + 1) * 128],
                                     rhs=h1[kh][:, :],
                                     start=(kh == 0), stop=(kh == 1))
                h_sb = scratch.tile([128, b_loc], F32, tag="h2sb")
                nc.scalar.activation(h_sb[:, :], h_ps[:, :], ACTF.Relu,
                                     bias=b2c[:, rh:rh + 1])
                h2.append(h_sb)
            q_ps = ps_c.tile([1, b_loc], F32, tag="cps")
            for kh in range(2):
                nc.tensor.matmul(out=q_ps[:, :], lhsT=w3T[:, kh:kh + 1],
                                 rhs=h2[kh][:, :], start=(kh == 0), stop=(kh == 1))
            q_row = scratch.tile([1, b_loc], F32, tag="qrow")
            nc.scalar.activation(q_row[:, :], q_ps[:, :], ACTF.Identity,
                                 bias=b3c[:, :])
            nc.sync.dma_start(out_d[h:h + 1, :], q_row[:, :])

    nc.compile()
    return nc


def _shard_inputs(inputs, b_loc=B_LOC):
    """Full inputs -> list of per-core in_maps."""
    gb = _make_gb()
    perm8 = _make_perm8()
    maps = []
    for c in range(NCORES):
        sl = slice(c * b_loc, (c + 1) * b_loc)
        m = {
            "local_states": np.ascontiguousarray(
                inputs["local_states"][sl].reshape(b_loc * L, L_DIM)),
            "global_states": np.ascontiguousarray(inputs["global_states"][sl]),
            "actions": np.ascontiguousarray(inputs["actions"][sl]),
            "gb": gb,
            "perm8": perm8,
        }
        for k in ("W_w", "W_b", "U_w", "U_b", "att_b",
                  "l1_w", "l1_b", "l2_w", "l2_b", "l3_w", "l3_b",
                  "l4_w", "l4_b", "l5_w", "l5_b", "l6_w", "l6_b"):
            m[k] = np.ascontiguousarray(np.asarray(inputs[k], np.float32))
        m["att_w"] = np.ascontiguousarray(
            np.asarray(inputs["att_w"], np.float32).reshape(1, 2 * HID))
        maps.append(m)
    return maps


_CACHE = {}


def kernel(**inputs) -> np.ndarray:
    from concourse.bass_utils import run_bass_kernel_spmd

    inputs = {k: np.asarray(v, np.float32) for k, v in inputs.items()}
    if "nc" not in _CACHE:
        _CACHE["nc"] = build_bass()
    nc = _CACHE["nc"]
    maps = _shard_inputs(inputs)
    res = run_bass_kernel_spmd(nc, maps, list(range(NCORES)))
    outs = [res.results[c]["out"] for c in range(NCORES)]  # each [2, B_LOC]
    q = np.concatenate(outs, axis=1)  # [2, B]
    return q.reshape(2, B, 1).astype(np.float32)


# revision 18
# speedup vs baseline: 1.8450x; 1.3100x over previous
"""Trainium2 Bass kernel for the Critic (gnn_message_passing) problem.

Math (per sample b):
  wg   = W_w @ g + W_b                                  [32]
  score_l = lrelu(x_l . v + c_b),  v = U_w^T a2,  c_b = a1.wg + att_b + U_b.a2
  score_g = lrelu((a1+a2).wg + att_b)
  total = score_g + sum_l score_l
  l_part = (U_w @ m_b + U_b * s_b) / total   with m_b = sum_l score_l x_l, s_b = sum_l score_l
  g_part = (score_g / total) * wg
  sa = [relu(g_part); relu(l_part); action]            [128]
  q_h = l3 @ relu(l2 @ relu(l1 @ sa + b1) + b2) + b3   (two heads)

Layout strategy (one NeuronCore handles B_LOC=512 samples, pure data parallel x8):
  Tokens are laid out BLOCKED: within a 16-sample chunk (3200 tokens), SBUF
  partition p holds tokens [25p, 25p+25).  Since 200 = 8*25, partition p
  always belongs to sample p//8 - no sample straddles a partition, so the
  attention bias c is a per-partition scalar and no boundary masking exists.
  DMA moves 12.8KB contiguous per partition per chunk.

  Per chunk (25 column-tiles of [128, 128]):
   - mult x*v: one big GPSIMD op (tiles 0:G) + one big DVE op (tiles G:25)
   - reduce to t[128,25]: one segmented DVE tensor_reduce (tiles 0:R) +
     per-tile ACT accum reduces (tiles R:25)
   - score = Lrelu(t + c) in one ACT op (c per-partition bias)
   - S_all[p,t',g] = score[p,t'] * [p//8==g]: one DVE mult with the
     block mask; lhsT of the m-matmul.
   - m^T[16,129] accumulated in PSUM over 25 matmuls: lhsT=S_all[:,t',:]
     (16 cols, cheap LDWEIGHTS), rhs=[x_tile | ones] (s_b rides along as
     column 128).
  At the end m (sample-major) is PE-transposed to feature-major mT for the
  combine; heads run feature-major on PE as in the reference mapping.
"""
import os
import sys

sys.path.insert(0, "/opt/trn_rl_repo")

from contextlib import ExitStack

import numpy as np

import concourse.bass as bass
import concourse.tile as tile
from concourse import bacc
from concourse import mybir

F32 = mybir.dt.float32
AF = mybir.AluOpType
ACTF = mybir.ActivationFunctionType

G_DIM, L_DIM, A_DIM, HID = 256, 128, 64, 32
B, L = 4096, 200
NCORES = 8
B_LOC = B // NCORES          # 512 samples per core
PERIOD = 25                  # column-tiles per chunk; 16 samples per chunk
SAMP = 16                    # samples per chunk
XW = 132                     # x_ch row stride in elems (129 used, pad 3)
G_TILES = 15                 # tiles whose x*v mult runs on GPSIMD
R_TILES = 20                 # tiles reduced by the segmented DVE reduce


def _make_gb():
    gb = np.zeros((128, SAMP), np.float32)
    for p in range(128):
        gb[p, p // 8] = 1.0
    return gb


def _make_perm8():
    # perm[q, 128k+p] = 1 iff q == 16k + p//8; used as lhsT so that
    # (perm_k)^T @ c_colblk gives c_all[p] = c[16k + p//8]
    m = np.zeros((128, 8 * 128), np.float32)
    for k in range(8):
        for p in range(128):
            m[16 * k + p // 8, 128 * k + p] = 1.0
    return m


def build_bass(b_loc=B_LOC):
    tok = b_loc * L
    nchunk = tok // (PERIOD * 128)     # 32
    nblk = nchunk // 8                 # 4 sample-blocks of 128

    nc = bacc.Bacc()

    ls = nc.dram_tensor("local_states", [tok, L_DIM], F32, kind="ExternalInput")
    gs = nc.dram_tensor("global_states", [b_loc, G_DIM], F32, kind="ExternalInput")
    ac = nc.dram_tensor("actions", [b_loc, A_DIM], F32, kind="ExternalInput")
    Ww = nc.dram_tensor("W_w", [HID, G_DIM], F32, kind="ExternalInput")
    Wb = nc.dram_tensor("W_b", [HID], F32, kind="ExternalInput")
    Uw = nc.dram_tensor("U_w", [HID, L_DIM], F32, kind="ExternalInput")
    Ub = nc.dram_tensor("U_b", [HID], F32, kind="ExternalInput")
    attw = nc.dram_tensor("att_w", [1, 2 * HID], F32, kind="ExternalInput")
    attb = nc.dram_tensor("att_b", [1], F32, kind="ExternalInput")
    heads = []
    for h, names in enumerate((("l1", "l2", "l3"), ("l4", "l5", "l6"))):
        w1 = nc.dram_tensor(f"{names[0]}_w", [256, 128], F32, kind="ExternalInput")
        b1 = nc.dram_tensor(f"{names[0]}_b", [256], F32, kind="ExternalInput")
        w2 = nc.dram_tensor(f"{names[1]}_w", [256, 256], F32, kind="ExternalInput")
        b2 = nc.dram_tensor(f"{names[1]}_b", [256], F32, kind="ExternalInput")
        w3 = nc.dram_tensor(f"{names[2]}_w", [1, 256], F32, kind="ExternalInput")
        b3 = nc.dram_tensor(f"{names[2]}_b", [1], F32, kind="ExternalInput")
        heads.append((w1, b1, w2, b2, w3, b3))
    gb_d = nc.dram_tensor("gb", [128, SAMP], F32, kind="ExternalInput")
    perm_d = nc.dram_tensor("perm8", [128, 8 * 128], F32, kind="ExternalInput")
    out_d = nc.dram_tensor("out", [2, b_loc], F32, kind="ExternalOutput")

    nb = b_loc // 128

    with tile.TileContext(nc) as tc, ExitStack() as ctx:
        P = ctx.enter_context(tc.tile_pool(name="persist", bufs=1))
        scratch = ctx.enter_context(tc.tile_pool(name="scratch", bufs=2))
        ctxA = ctx.enter_context(ExitStack())
        ps_t = ctxA.enter_context(tc.tile_pool(name="ps_t", bufs=2, space="PSUM"))

        # ---------------- Phase A: constants & small precompute ----------------
        from concourse.masks import make_identity

        ident = P.tile([128, 128], F32, tag="ident")
        make_identity(nc, ident[:, :])
        ones_row = P.tile([1, 128], F32, tag="onesr")
        nc.vector.memset(ones_row[:, :], 1.0)
        gb_sb = P.tile([128, SAMP], F32, tag="gb")
        nc.sync.dma_start(gb_sb[:, :], gb_d[:, :])
        perm_sb = P.tile([128, 8 * 128], F32, tag="perm")
        nc.sync.dma_start(perm_sb[:, :], perm_d[:, :])

        def transpose_to_sbuf(dst_ap, src_ap):
            """dst[f, p] = src[p, f] via PE transpose + ACT copy out of PSUM."""
            pp, ff = src_ap.shape
            t_ps = ps_t.tile([128, 128], F32, tag="tps")
            nc.tensor.transpose(t_ps[0:ff, 0:pp], src_ap, ident[0:pp, 0:pp])
            nc.scalar.copy(dst_ap, t_ps[0:ff, 0:pp])

        # small weights
        Ww_sb = P.tile([HID, G_DIM], F32, tag="Ww")
        nc.sync.dma_start(Ww_sb[:, :], Ww[:, :])
        Wb_sb = P.tile([HID, 1], F32, tag="Wb")
        nc.sync.dma_start(Wb_sb[:, :], Wb[:][:, None])
        Uw_sb = P.tile([HID, L_DIM], F32, tag="Uw")
        nc.sync.dma_start(Uw_sb[:, :], Uw[:, :])
        Ub_col = P.tile([HID, 1], F32, tag="Ubc")
        nc.sync.dma_start(Ub_col[:, :], Ub[:][:, None])
        Ub_row = P.tile([1, HID], F32, tag="Ubr")
        nc.sync.dma_start(Ub_row[:, :], Ub[:][None, :])
        a1_sb = P.tile([HID, 1], F32, tag="a1")
        nc.sync.dma_start(a1_sb[:, :], attw[0, 0:HID][:, None])
        a2_sb = P.tile([HID, 1], F32, tag="a2")
        nc.sync.dma_start(a2_sb[:, :], attw[0, HID:2 * HID][:, None])
        attb_sb = P.tile([1, 1], F32, tag="attb")
        nc.sync.dma_start(attb_sb[:, :], attb[:][None, :])

        WwT = []  # W_w^T in [128, HID] chunks over G_DIM
        for g in range(G_DIM // 128):
            w = P.tile([128, HID], F32, tag=f"WwT{g}")
            transpose_to_sbuf(w[:, :], Ww_sb[:, g * 128:(g + 1) * 128])
            WwT.append(w)
        UwT = P.tile([L_DIM, HID], F32, tag="UwT")
        transpose_to_sbuf(UwT[:, :], Uw_sb[:, :])

        # gT: global_states^T  [G_DIM partition-chunks][128, b_loc]
        gT = []
        for g in range(G_DIM // 128):
            t = P.tile([128, b_loc], F32, tag=f"gT{g}")
            gT.append(t)
        for bb in range(nb):
            g_nat = scratch.tile([128, G_DIM], F32, tag="gnat")
            nc.sync.dma_start(g_nat[:, :], gs[bb * 128:(bb + 1) * 128, :])
            for g in range(G_DIM // 128):
                transpose_to_sbuf(gT[g][:, bb * 128:(bb + 1) * 128],
                                  g_nat[:, g * 128:(g + 1) * 128])

        # wg^T [HID, b_loc]
        wgT_ps = ps_t.tile([HID, b_loc], F32, tag="wps")
        for g in range(G_DIM // 128):
            nc.tensor.matmul(out=wgT_ps[:, :], lhsT=WwT[g][:, :], rhs=gT[g][:, :],
                             start=(g == 0), stop=(g == G_DIM // 128 - 1))
        wgT = P.tile([HID, b_loc], F32, tag="wgT")
        nc.scalar.activation(wgT[:, :], wgT_ps[:, :], ACTF.Identity, bias=Wb_sb[:, :])

        # v_row [1, 128] = a2^T U_w ;  v_rep [128, 128] = ones (x) v_row
        v_ps = ps_t.tile([1, L_DIM], F32, tag="tps")
        nc.tensor.matmul(out=v_ps[:, :], lhsT=a2_sb[:, :], rhs=Uw_sb[:, :])
        v_row = P.tile([1, L_DIM], F32, tag="vrow")
        nc.scalar.copy(v_row[:, :], v_ps[:, :])
        vrep_ps = ps_t.tile([128, 128], F32, tag="tps")
        nc.tensor.matmul(out=vrep_ps[:, :], lhsT=ones_row[:, :], rhs=v_row[:, :])
        v_rep = P.tile([128, 128], F32, tag="vrep")
        nc.scalar.copy(v_rep[:, :], vrep_ps[:, :])

        # cconst = Ub.a2 + att_b, replicated to [128, 1]
        uba2_ps = ps_t.tile([1, 1], F32, tag="tps")
        nc.tensor.matmul(out=uba2_ps[:, :], lhsT=Ub_col[:, :], rhs=a2_sb[:, :])
        cconst = P.tile([1, 1], F32, tag="cconst")
        nc.vector.tensor_tensor(out=cconst[:, :], in0=uba2_ps[:, :], in1=attb_sb[:, :],
                                op=AF.add)
        ccol_ps = ps_t.tile([128, 1], F32, tag="tps")
        nc.tensor.matmul(out=ccol_ps[:, :], lhsT=ones_row[:, :], rhs=cconst[:, :])
        cconst_col = P.tile([128, 1], F32, tag="cconstc")
        nc.scalar.copy(cconst_col[:, :], ccol_ps[:, :])

        # c_colblk [128, nblk]: c_lin of sample 128b+q at (q, b) (no const yet)
        c_colblk = P.tile([128, nblk], F32, tag="ccb")
        for b in range(nblk):
            cb_ps = ps_t.tile([128, 1], F32, tag="tps")
            nc.tensor.matmul(out=cb_ps[:, :], lhsT=wgT[:, b * 128:(b + 1) * 128],
                             rhs=a1_sb[:, :])
            nc.scalar.copy(c_colblk[:, b:b + 1], cb_ps[:, :])

        # c_all [128, nchunk]: c (incl const) for partition p in chunk ch
        call_ps = ps_t.tile([128, nchunk], F32, tag="cps2")
        for k in range(8):
            nc.tensor.matmul(out=call_ps[:, k:nchunk:8],
                             lhsT=perm_sb[:, k * 128:(k + 1) * 128],
                             rhs=c_colblk[:, :], start=True, stop=True,
                             skip_group_check=True)
        c_all = P.tile([128, nchunk], F32, tag="call")
        nc.scalar.activation(c_all[:, :], call_ps[:, :], ACTF.Identity,
                             bias=cconst_col[:, :])

        # sg_raw [1, b_loc] = lrelu((a1+a2).wg + att_b)
        a12 = P.tile([HID, 1], F32, tag="a12")
        nc.vector.tensor_tensor(out=a12[:, :], in0=a1_sb[:, :], in1=a2_sb[:, :],
                                op=AF.add)
        sg_ps = ps_t.tile([1, b_loc], F32, tag="tps")
        nc.tensor.matmul(out=sg_ps[:, :], lhsT=a12[:, :], rhs=wgT[:, :])
        sg_raw = P.tile([1, b_loc], F32, tag="sg_raw")
        nc.scalar.activation(sg_raw[:, :], sg_ps[:, :], ACTF.Lrelu,
                             bias=attb_sb[:, :], alpha=0.01)

        # actions^T into sa^T[64:128]
        saT = P.tile([128, b_loc], F32, tag="saT")
        for bb in range(nb):
            a_nat = scratch.tile([128, A_DIM], F32, tag="anat")
            nc.sync.dma_start(a_nat[:, :], ac[bb * 128:(bb + 1) * 128, :])
            transpose_to_sbuf(saT[2 * HID:2 * HID + A_DIM, bb * 128:(bb + 1) * 128],
                              a_nat[:, :])

        # MLP head weights, transposed
        head_sb = []
        for (w1, b1, w2, b2, w3, b3) in heads:
            w1_nat = scratch.tile([128, 128], F32, tag="w1nat")
            w1T = P.tile([128, 256], F32, tag=f"w1T{len(head_sb)}")
            for rh in range(2):
                nc.sync.dma_start(w1_nat[:, :], w1[rh * 128:(rh + 1) * 128, :])
                transpose_to_sbuf(w1T[:, rh * 128:(rh + 1) * 128], w1_nat[:, :])
            w2T = [P.tile([128, 256], F32, tag=f"w2T{len(head_sb)}_{kh}",
                          name=f"w2T{len(head_sb)}_{kh}")
                   for kh in range(2)]
            for rh in range(2):
                for kh in range(2):
                    w2_nat = scratch.tile([128, 128], F32, tag="w2nat")
                    nc.sync.dma_start(
                        w2_nat[:, :],
                        w2[rh * 128:(rh + 1) * 128, kh * 128:(kh + 1) * 128])
                    transpose_to_sbuf(w2T[kh][:, rh * 128:(rh + 1) * 128],
                                      w2_nat[:, :])
            w3T = P.tile([128, 2], F32, tag=f"w3T{len(head_sb)}")
            for kh in range(2):
                nc.sync.dma_start(w3T[:, kh:kh + 1],
                                  w3[0, kh * 128:(kh + 1) * 128][:, None])
            b1c = P.tile([128, 2], F32, tag=f"b1c{len(head_sb)}")
            b2c = P.tile([128, 2], F32, tag=f"b2c{len(head_sb)}")
            for rh in range(2):
                nc.sync.dma_start(b1c[:, rh:rh + 1],
                                  b1[rh * 128:(rh + 1) * 128][:, None])
                nc.sync.dma_start(b2c[:, rh:rh + 1],
                                  b2[rh * 128:(rh + 1) * 128][:, None])
            b3c = P.tile([1, 1], F32, tag=f"b3c{len(head_sb)}")
            nc.sync.dma_start(b3c[:, :], b3[:][None, :])
            head_sb.append((w1T, w2T, w3T, b1c, b2c, b3c))

        ctxA.close()

        # ---------------- Phase B: main token stream ----------------
        ctxB = ctx.enter_context(ExitStack())
        xpool = ctx.enter_context(tc.tile_pool(name="xchunk", bufs=3))
        ppool = ctx.enter_context(tc.tile_pool(name="prod", bufs=2))
        tpool = ctx.enter_context(tc.tile_pool(name="tbuf", bufs=2))
        spool = ctx.enter_context(tc.tile_pool(name="sall", bufs=2))
        jpool = ctx.enter_context(tc.tile_pool(name="junk", bufs=2))
        ps_m = ctxB.enter_context(tc.tile_pool(name="ps_m", bufs=2, space="PSUM"))

        m_nat = [P.tile([128, XW], F32, tag=f"mnat{b}", name=f"mnat{b}")
                 for b in range(nblk)]
        mT = P.tile([L_DIM, b_loc], F32, tag="mT")
        s_row = P.tile([1, b_loc], F32, tag="srow")

        # software-pipelined emission: stage A (DMA + mults) of chunk c+1 is
        # emitted before stage B (reduce..matmuls) of chunk c so in-order
        # engines always have next-chunk work queued ahead of cross-engine
        # waits; the PSUM copy-out lags one more chunk.
        live = {}

        def emit_A(ch):
            x_ch = xpool.tile([128, PERIOD * XW], F32, tag="xch")
            x3 = x_ch[:, :].rearrange("p (j d) -> p j d", d=XW)
            src = ls[ch * PERIOD * 128:(ch + 1) * PERIOD * 128, :]
            nc.sync.dma_start(x3[:, :, 0:L_DIM],
                              src.rearrange("(p t) d -> p t d", t=PERIOD))
            if ch < 3:
                # ones column; x pool cycles 3 buffers, col 128 is never
                # overwritten so 3 writes cover the whole run
                nc.vector.memset(x3[:, :, 128:129], 1.0)

            prod = ppool.tile([128, PERIOD * 128], F32, tag="pr")
            p3 = prod[:, :].rearrange("p (j d) -> p j d", d=128)
            nc.gpsimd.tensor_tensor(
                out=p3[:, 0:G_TILES, :],
                in0=x3[:, 0:G_TILES, 0:128],
                in1=v_rep[:, None, :].broadcast_to((128, G_TILES, 128)),
                op=AF.mult)
            nc.vector.tensor_tensor(
                out=p3[:, G_TILES:PERIOD, :],
                in0=x3[:, G_TILES:PERIOD, 0:128],
                in1=v_rep[:, None, :].broadcast_to((128, PERIOD - G_TILES, 128)),
                op=AF.mult)
            live[ch] = {"x3": x3, "p3": p3}

        def emit_B(ch):
            x3, p3 = live[ch]["x3"], live[ch]["p3"]
            t_buf = tpool.tile([128, PERIOD], F32, tag="tb")
            nc.vector.tensor_reduce(out=t_buf[:, 0:R_TILES],
                                    in_=p3[:, 0:R_TILES, :],
                                    axis=mybir.AxisListType.X, op=AF.add)
            junk = jpool.tile([128, 128], F32, tag="jk")
            for j in range(R_TILES, PERIOD):
                nc.scalar.activation(junk[:, :], p3[:, j, :], ACTF.Identity,
                                     accum_out=t_buf[:, j:j + 1])

            score = tpool.tile([128, PERIOD], F32, tag="sc")
            nc.scalar.activation(score[:, :], t_buf[:, :], ACTF.Lrelu,
                                 bias=c_all[:, ch:ch + 1], alpha=0.01)

            s_all = spool.tile([128, PERIOD * SAMP], F32, tag="sa")
            sa3 = s_all[:, :].rearrange("p (j g) -> p j g", g=SAMP)
            nc.vector.tensor_tensor(
                out=sa3[:, :, :],
                in0=score[:, :, None].broadcast_to((128, PERIOD, SAMP)),
                in1=gb_sb[:, None, :].broadcast_to((128, PERIOD, SAMP)),
                op=AF.mult)

            m_ps = ps_m.tile([SAMP, 129], F32, tag="mps")
            for j in range(PERIOD):
                nc.tensor.matmul(out=m_ps[:, :], lhsT=sa3[:, j, :],
                                 rhs=x3[:, j, 0:129],
                                 start=(j == 0), stop=(j == PERIOD - 1))
            live[ch]["m_ps"] = m_ps

        def emit_copy(ch):
            # engines can't write at 16-partition offsets; stage at partition 0
            # and let DMA do the partition shift
            m_ps = live[ch].pop("m_ps")
            m_stage = jpool.tile([SAMP, 132], F32, tag="mstg")
            nc.scalar.copy(m_stage[:, 0:129], m_ps[:, :])
            blk, row = ch // 8, (ch % 8) * SAMP
            nc.sync.dma_start(m_nat[blk][row:row + SAMP, 0:129],
                              m_stage[:, 0:129])
            del live[ch]

        emit_A(0)
        for ch in range(1, nchunk):
            emit_A(ch)
            emit_B(ch - 1)
            if ch >= 2:
                emit_copy(ch - 2)
        emit_B(nchunk - 1)
        emit_copy(nchunk - 2)
        emit_copy(nchunk - 1)

        # m_nat [sample, feat] -> mT [feat, sample]; s column -> s_row
        ps_f = ctxB.enter_context(tc.tile_pool(name="ps_f", bufs=2, space="PSUM"))
        s_nat = P.tile([128, nblk], F32, tag="snat")
        for b in range(nblk):
            f_ps = ps_f.tile([128, 128], F32, tag="fps")
            nc.tensor.transpose(f_ps[:, :], m_nat[b][:, 0:128], ident[:, :])
            nc.scalar.copy(mT[:, b * 128:(b + 1) * 128], f_ps[:, :])
            nc.vector.tensor_copy(s_nat[:, b:b + 1], m_nat[b][:, 128:129])
        sn_ps = ps_f.tile([nblk, 128], F32, tag="fps")
        nc.tensor.transpose(sn_ps[:, :], s_nat[:, :], ident[:, :])
        s_nat4 = P.tile([nblk, 128], F32, tag="snat4")
        nc.scalar.copy(s_nat4[:, :], sn_ps[:, :])
        for b in range(nblk):
            nc.sync.dma_start(s_row[0:1, b * 128:(b + 1) * 128],
                              s_nat4[b:b + 1, :])

        ctxB.close()

        # ---------------- Phase C: combine + heads ----------------
        ps_c = ctx.enter_context(tc.tile_pool(name="ps_c", bufs=4, space="PSUM"))

        total = P.tile([1, b_loc], F32, tag="total")
        nc.vector.tensor_tensor(out=total[:, :], in0=sg_raw[:, :], in1=s_row[:, :],
                                op=AF.add)
        recip = P.tile([1, b_loc], F32, tag="recip")
        nc.vector.reciprocal(recip[:, :], total[:, :])
        gn_row = P.tile([1, b_loc], F32, tag="gn")
        nc.vector.tensor_tensor(out=gn_row[:, :], in0=sg_raw[:, :], in1=recip[:, :],
                                op=AF.mult)

        r32_ps = ps_c.tile([HID, b_loc], F32, tag="cps")
        nc.tensor.matmul(out=r32_ps[:, :], lhsT=ones_row[0:1, 0:HID], rhs=recip[:, :])
        r32 = P.tile([HID, b_loc], F32, tag="r32")
        nc.scalar.copy(r32[:, :], r32_ps[:, :])
        g32_ps = ps_c.tile([HID, b_loc], F32, tag="cps")
        nc.tensor.matmul(out=g32_ps[:, :], lhsT=ones_row[0:1, 0:HID], rhs=gn_row[:, :])
        g32 = P.tile([HID, b_loc], F32, tag="g32")
        nc.scalar.copy(g32[:, :], g32_ps[:, :])

        lT_ps = ps_c.tile([HID, b_loc], F32, tag="cps")
        nc.tensor.matmul(out=lT_ps[:, :], lhsT=UwT[:, :], rhs=mT[:, :],
                         start=True, stop=False)
        nc.tensor.matmul(out=lT_ps[:, :], lhsT=Ub_row[:, :], rhs=s_row[:, :],
                         start=False, stop=True)

        lnorm = P.tile([HID, b_loc], F32, tag="lnorm")
        nc.vector.tensor_tensor(out=lnorm[:, :], in0=lT_ps[:, :], in1=r32[:, :],
                                op=AF.mult)
        gpart = P.tile([HID, b_loc], F32, tag="gpart")
        nc.vector.tensor_tensor(out=gpart[:, :], in0=wgT[:, :], in1=g32[:, :],
                                op=AF.mult)
        nc.scalar.activation(saT[0:HID, :], gpart[:, :], ACTF.Relu)
        nc.scalar.activation(saT[HID:2 * HID, :], lnorm[:, :], ACTF.Relu)

        for h, (w1T, w2T, w3T, b1c, b2c, b3c) in enumerate(head_sb):
            h1 = []
            for rh in range(2):
                h_ps = ps_c.tile([128, b_loc], F32, tag="cps")
                nc.tensor.matmul(out=h_ps[:, :], lhsT=w1T[:, rh * 128:(rh + 1) * 128],
                                 rhs=saT[:, :])
                h_sb = scratch.tile([128, b_loc], F32, tag="h1sb")
                nc.scalar.activation(h_sb[:, :], h_ps[:, :], ACTF.Relu,
                                     bias=b1c[:, rh:rh + 1])
                h1.append(h_sb)
            h2 = []
            for rh in range(2):
                h_ps = ps_c.tile([128, b_loc], F32, tag="cps")
                for kh in range(2):
                    nc.tensor.matmul(out=h_ps[:, :],
                                     lhsT=w2T[kh][:, rh * 128:(rh + 1) * 128],
                                     rhs=h1[kh][:, :],
                                     start=(kh == 0), stop=(kh == 1))
                h_sb = scratch.tile([128, b_loc], F32, tag="h2sb")
                nc.scalar.activation(h_sb[:, :], h_ps[:, :], ACTF.Relu,
                                     bias=b2c[:, rh:rh + 1])
                h2.append(h_sb)
            q_ps = ps_c.tile([1, b_loc], F32, tag="cps")
            for kh in range(2):
                nc.tensor.matmul(out=q_ps[:, :], lhsT=w3T[:, kh:kh + 1],
                                 rhs=h2[kh][:, :], start=(kh == 0), stop=(kh == 1))
            q_row = scratch.tile([1, b_loc], F32, tag="qrow")
            nc.scalar.activation(q_row[:, :], q_ps[:, :], ACTF.Identity,
                                 bias=b3c[:, :])
            nc.sync.dma_start(out_d[h:h + 1, :], q_row[:, :])

    nc.compile()
    return nc


def _shard_inputs(inputs, b_loc=B_LOC):
    """Full inputs -> list of per-core in_maps."""
    gb = _make_gb()
    perm8 = _make_perm8()
    maps = []
    for c in range(NCORES):
        sl = slice(c * b_loc, (c + 1) * b_loc)
        m = {
            "local_states": np.ascontiguousarray(
                inputs["local_states"][sl].reshape(b_loc * L, L_DIM)),
            "global_states": np.ascontiguousarray(inputs["global_states"][sl]),
            "actions": np.ascontiguousarray(inputs["actions"][sl]),
            "gb": gb,
            "perm8": perm8,
        }
        for k in ("W_w", "W_b", "U_w", "U_b", "att_b",
                  "l1_w", "l1_b", "l2_w", "l2_b", "l3_w", "l3_b",
                  "l4_w", "l4_b", "l5_w", "l5_b", "l6_w", "l6_b"):
            m[k] = np.ascontiguousarray(np.asarray(inputs[k], np.float32))
        m["att_w"] = np.ascontiguousarray(
            np.asarray(inputs["att_w"], np.float32).reshape(1, 2 * HID))
        maps.append(m)
    return maps


_CACHE = {}


def kernel(**inputs) -> np.ndarray:
    from concourse.bass_utils import run_bass_kernel_spmd

    inputs = {k: np.asarray(v, np.float32) for k, v in inputs.items()}
    if "nc" not in _CACHE:
        _CACHE["nc"] = build_bass()
    nc = _CACHE["nc"]
    maps = _shard_inputs(inputs)
    res = run_bass_kernel_spmd(nc, maps, list(range(NCORES)))
    outs = [res.results[c]["out"] for c in range(NCORES)]  # each [2, B_LOC]
    q = np.concatenate(outs, axis=1)  # [2, B]
    return q.reshape(2, B, 1).astype(np.float32)
